# revision 1
# baseline (speedup 1.0000x reference)
"""GQA sparse attention (packed seqs + sliding window + RoPE) on 8 Trainium2 cores.

Sharding: tensor-parallel over heads. Each of the 8 cores owns 4 Q-heads and
their single shared KV-head (GQA groups stay intact): wq columns
[h*512:(h+1)*512], wk/wv columns [h*128:(h+1)*128], wo rows [h*512:(h+1)*512].
Every core computes a full [S, DIM] partial of the output projection; the host
sums the 8 partials.

The mask never reaches the device: seqlens [1024, 512, 512] with causal +
sliding-window 1024 reduce to block-causal over 128-blocks within each
sequence (the window can never truncate since max causal span == 1024), plus
a causal bias on the diagonal 128x128 blocks.

Per-core dataflow (all matmuls bf16 with fp32 PSUM accumulation):
  qkv:   psum[s,768] = sum_cb xT[cb,si].T @ wqkv[cb]      (weights resident)
  rope:  strided DVE ops on the psum, [s,d] layout, fp32 in / bf16 out
  qT/kT: PE transposes of the roped blocks
  scores(T): psum[sk, sq_span] = kT_blk.T @ qT[h]         (block-causal spans)
  p:     exp(scores + diag_bias) -> pT buffer, bf16       (no max subtraction:
         scores are O(5), exp is safe in fp32)
  pv:    psum[sq, 129] = sum_kj pT_blk.T @ [v_blk | ones] (sums ride along)
  out:   attn = pv[:, :128] * recip(pv[:, 128]),  transpose -> attnT
  wo:    psum[c',s] = sum_db wo[db,cp].T @ attnT[db]      -> DRAM [4096, 2048]
"""

import os

os.environ.setdefault("JAX_PLATFORMS", "axon")

import numpy as np

import concourse.bass as bass
import concourse.mybir as mybir
import concourse.tile as tile
from concourse import bacc
from concourse.bass_utils import run_bass_kernel_spmd

# ---- problem constants (hardcoded per harness contract) ----
DIM = 4096
N_HEADS = 32
N_KV_HEADS = 8
HEAD_DIM = 128
SEQLENS = [1024, 512, 512]
S = 2048
N_CORES = 8
HPC = N_HEADS // N_CORES          # q heads per core = 4
QW = HPC * HEAD_DIM               # per-core q width = 512
KW = HEAD_DIM                     # per-core k/v width = 128
B = 128                           # block size
NSB = S // B                      # 16 seq blocks
NCB = DIM // B                    # 32 contraction blocks
SEQ_BLOCKS = []                   # [(start_blk, end_blk)] per packed sequence
_b = 0
for _l in SEQLENS:
    SEQ_BLOCKS.append((_b, _b + _l // B))
    _b += _l // B

# pT buffer layout: for each kj, columns [offs[kj] : offs[kj]+span(kj)) hold
# p.T for queries sq in [kj*B, seq_end)
_SPANS = {}
_OFFS = {}
_off = 0
for _s0, _s1 in SEQ_BLOCKS:
    for _kj in range(_s0, _s1):
        _SPANS[_kj] = (_s1 - _kj) * B
        _OFFS[_kj] = _off
        _off += _SPANS[_kj]
PT_COLS = _off                    # 7168

F32 = mybir.dt.float32
BF16 = mybir.dt.bfloat16

_PROGRAM = None


def _build_program():
    nc = bacc.Bacc(trn_type="TRN2")

    xt_h = nc.declare_dram_parameter("xt", [NSB, B, DIM], BF16, isOutput=False)
    wqkv_h = nc.declare_dram_parameter("wqkv", [DIM, QW + 2 * KW], BF16, isOutput=False)
    wo_h = nc.declare_dram_parameter("wo", [QW, DIM], BF16, isOutput=False)
    cos_h = nc.declare_dram_parameter("cosr", [NSB, B, 2 * HEAD_DIM], F32, isOutput=False)
    sin_h = nc.declare_dram_parameter("sinr", [NSB, B, 2 * HEAD_DIM], F32, isOutput=False)
    dmask_h = nc.declare_dram_parameter("dmask", [B, B], F32, isOutput=False)
    ident_h = nc.declare_dram_parameter("ident", [B, B], BF16, isOutput=False)
    out_h = nc.declare_dram_parameter("outp", [DIM, S], BF16, isOutput=True)

    W768 = QW + 2 * KW  # 768
    Exp = mybir.ActivationFunctionType.Exp

    with tile.TileContext(nc) as tc:
        with (
            tc.tile_pool(name="consts", bufs=1) as cpool,
            tc.tile_pool(name="big", bufs=1) as bigp,
            tc.tile_pool(name="persist", bufs=1) as pers,
            tc.tile_pool(name="roam", bufs=4) as roam,
            tc.tile_pool(name="work", bufs=3) as work,
            tc.tile_pool(name="psum", bufs=2, space="PSUM") as psum,
        ):
            # first x block before the big weight DMAs so PE starts early
            xt0_t = work.tile([B, DIM], BF16, tag="xt", bufs=3)
            nc.sync.dma_start(out=xt0_t[:], in_=xt_h[0])

            # ---- resident tensors ----
            # interleave early x blocks into the weight stream so si=1..3
            # can start before the full wqkv has landed
            early_xt = {}
            wqkv_sb = bigp.tile([B, NCB * W768], BF16, tag="big")
            for cb in range(NCB):
                nc.sync.dma_start(
                    out=wqkv_sb[:, cb * W768:(cb + 1) * W768],
                    in_=wqkv_h[cb * B:(cb + 1) * B, :],
                )
                if cb in (7, 15, 23):
                    si_pre = cb // 8 + 1
                    t = work.tile([B, DIM], BF16, tag="xt", bufs=3, name=f"xtp{si_pre}")
                    nc.sync.dma_start(out=t[:], in_=xt_h[si_pre])
                    early_xt[si_pre] = t

            # ---- constants (after weights: nothing needs them until rope) ----
            ident_sb = cpool.tile([B, B], BF16)
            nc.sync.dma_start(out=ident_sb[:], in_=ident_h[:])
            dmask_sb = cpool.tile([B, B], F32)
            nc.sync.dma_start(out=dmask_sb[:], in_=dmask_h[:])

            attnT_sb = pers.tile([B, HPC * S], BF16)   # per head h: cols [h*S, (h+1)*S)
            qT_sb = pers.tile([B, HPC * S], BF16)      # per head h: cols [h*S, (h+1)*S)
            kT_sb = pers.tile([B, S], BF16)
            vaug_sb = pers.tile([B, NSB * 129], BF16)  # per kj: [v_blk | ones]

            # scores + exp for one (head, kj) block-row
            pTs = []

            def _scores(h, kj, s1, chunked=False):
                pT = pTs[h]
                span = (s1 - kj) * B
                if chunked:
                    # phase-A-overlapped variant: 512-col chunks through tag B
                    # (PV's tag, idle during phase A) so the qkv psum pipeline
                    # in tag A is never paced by exp latency
                    for part in range(0, span, 512):
                        n = min(512, span - part)
                        ps_c = psum.tile([B, 512], F32, tag="B", bufs=2,
                                         name="ps_c")
                        nc.tensor.matmul(
                            ps_c[:, 0:n],
                            kT_sb[:, kj * B:(kj + 1) * B],
                            qT_sb[:, h * S + kj * B + part:
                                  h * S + kj * B + part + n],
                            start=True, stop=True,
                        )
                        if part == 0:
                            nc.vector.tensor_add(
                                ps_c[:, 0:B], ps_c[:, 0:B], dmask_sb[:]
                            )
                        nc.scalar.activation(
                            pT[:, _OFFS[kj] + part:_OFFS[kj] + part + n],
                            ps_c[:, 0:n], Exp
                        )
                    return
                ps_sc = psum.tile([B, 1024], F32, tag="A", bufs=2, name="ps_sc")
                for part in range(0, span, 512):
                    n = min(512, span - part)
                    nc.tensor.matmul(
                        ps_sc[:, part:part + n],
                        kT_sb[:, kj * B:(kj + 1) * B],
                        qT_sb[:, h * S + kj * B + part:
                              h * S + kj * B + part + n],
                        start=True, stop=True,
                    )
                # causal bias on the diagonal block
                nc.vector.tensor_add(ps_sc[:, 0:B], ps_sc[:, 0:B], dmask_sb[:])
                nc.scalar.activation(
                    pT[:, _OFFS[kj]:_OFFS[kj] + span], ps_sc[:, 0:span], Exp
                )

            # seq0/seq1 score groups overlap the back half of the qkv phase
            # (their qT/kT inputs are complete by then); seq2 runs after
            chains_done = set()
            sched = {si: [] for si in range(NSB)}
            for kj in range(0, 8):
                sched[8 + kj] = [(h, kj, 8) for h in range(HPC)]
            for kj, si in ((8, 12), (9, 13), (10, 14), (11, 15)):
                sched[si] += [(h, kj, 12) for h in range(HPC)]
            done = set()

            def _chains(scol, tp_tag="C"):
                for qi in range(scol * 4, scol * 4 + 4):
                    if qi in chains_done:
                        continue
                    _chain_qi(qi, tp_tag)

            def _chain_qi(qi, tp_tag="C", copy_act=False):
                    chains_done.add(qi)
                    s0, s1 = next(b for b in SEQ_BLOCKS if b[0] <= qi < b[1])
                    for h in range(HPC):
                        pT = pTs[h]
                        ps_pv = psum.tile([B, 129], F32, tag="B", bufs=2)
                        for kj in range(s0, qi + 1):
                            lhsT = pT[:, _OFFS[kj] + (qi - kj) * B:
                                      _OFFS[kj] + (qi - kj + 1) * B]
                            nc.tensor.matmul(
                                ps_pv[:], lhsT,
                                vaug_sb[:, kj * 129:(kj + 1) * 129],
                                start=(kj == s0), stop=(kj == qi),
                            )
                        pv_sb = work.tile([B, 129], F32, tag="pv", bufs=8)
                        nc.vector.tensor_copy(pv_sb[:], ps_pv[:])
                        rc = work.tile([B, 1], F32, tag="rc", bufs=8)
                        nc.vector.reciprocal(rc[:], pv_sb[:, 128:129])
                        at = work.tile([B, B], BF16, tag="at", bufs=8)
                        nc.vector.tensor_scalar_mul(at[:], pv_sb[:, 0:B], rc[:])
                        tp = psum.tile([B, B], BF16, tag=tp_tag, bufs=2)
                        nc.tensor.transpose(tp[:], at[:], ident_sb[:])
                        dst = attnT_sb[:, h * S + qi * B:h * S + (qi + 1) * B]
                        if copy_act:
                            nc.scalar.copy(dst, tp[:])
                        else:
                            nc.vector.tensor_copy(dst, tp[:])

            def _wo(scol, use_a=False):
                for cp in range(NCB):
                    # alternate accumulators across tags C and A (A is idle in
                    # the late passes) for a 4-deep pipeline
                    if use_a and cp % 2 == 1:
                        pso = psum.tile([B, 512], F32, tag="A", bufs=2,
                                        name="psoA")
                    elif use_a is None and cp >= 16 and cp % 2 == 1:
                        # late wo(0) groups: seq2 chains have drained tag B
                        pso = psum.tile([B, 512], F32, tag="B", bufs=2,
                                        name="psoB")
                    else:
                        pso = psum.tile([B, 512], F32, tag="C", bufs=2,
                                        name="pso")
                    for db in range(HPC):
                        nc.tensor.matmul(
                            pso[:],
                            wo_sb[:, db * DIM + cp * B:db * DIM + (cp + 1) * B],
                            attnT_sb[:, db * S + scol * 512:db * S + (scol + 1) * 512],
                            start=(db == 0), stop=(db == HPC - 1),
                        )
                    ot = work.tile([B, 512], BF16, tag="ot", bufs=8)
                    if cp % 2 == 0:
                        nc.scalar.copy(ot[:], pso[:])
                    else:
                        nc.vector.tensor_copy(ot[:], pso[:])
                    nc.sync.dma_start(
                        out=out_h[cp * B:(cp + 1) * B, scol * 512:(scol + 1) * 512],
                        in_=ot[:],
                    )

            # =========== Phase A: qkv projection + rope + transposes ===========
            for si in range(NSB):
                if si == 0:
                    xt_t = xt0_t
                elif si in early_xt:
                    xt_t = early_xt[si]
                else:
                    xt_t = work.tile([B, DIM], BF16, tag="xt", bufs=3)
                    nc.sync.dma_start(out=xt_t[:], in_=xt_h[si])
                # small rotating cos/sin tiles (dead after rope of this si)
                c_t = work.tile([B, 256], F32, tag="cs", bufs=3)
                nc.sync.dma_start(out=c_t[:], in_=cos_h[si])
                s_t = work.tile([B, 256], F32, tag="sn", bufs=3)
                nc.sync.dma_start(out=s_t[:], in_=sin_h[si])
                ps = psum.tile([B, W768], F32, tag="A", bufs=2)
                psQ = ps[:, 0:512]
                psKV = ps[:, 512:768]
                for cb in range(NCB):
                    lhsT = xt_t[:, cb * B:(cb + 1) * B]
                    nc.tensor.matmul(
                        psQ, lhsT, wqkv_sb[:, cb * W768:cb * W768 + 512],
                        start=(cb == 0), stop=(cb == NCB - 1),
                    )
                    nc.tensor.matmul(
                        psKV, lhsT,
                        wqkv_sb[:, cb * W768 + 512:cb * W768 + 768],
                        start=(cb == 0), stop=(cb == NCB - 1),
                    )

                cs = c_t[:]
                sn = s_t[:]

                # rope on q: [s, d] layout, channels interleaved (even, odd)
                q_t = work.tile([B, QW], BF16, tag="q", bufs=3)
                qe, qo = ps[:, 0:QW:2], ps[:, 1:QW:2]
                t1 = work.tile([B, 256], F32, tag="t1", bufs=2)
                t2 = work.tile([B, 256], F32, tag="t2", bufs=2)
                t3 = work.tile([B, 256], F32, tag="t3", bufs=2)
                t4 = work.tile([B, 256], F32, tag="t4", bufs=2)
                nc.vector.tensor_mul(t1[:], qe, cs)
                nc.vector.tensor_mul(t2[:], qo, sn)
                nc.vector.tensor_sub(q_t[:, 0:QW:2], t1[:], t2[:])
                nc.vector.tensor_mul(t3[:], qe, sn)
                nc.vector.tensor_mul(t4[:], qo, cs)
                nc.vector.tensor_add(q_t[:, 1:QW:2], t3[:], t4[:])

                # rope on k
                k_t = work.tile([B, KW], BF16, tag="k", bufs=3)
                ke, ko = ps[:, 512:640:2], ps[:, 513:640:2]
                c64, s64 = c_t[:, 0:64], s_t[:, 0:64]
                u1 = work.tile([B, 64], F32, tag="u1", bufs=2)
                u2 = work.tile([B, 64], F32, tag="u2", bufs=2)
                u3 = work.tile([B, 64], F32, tag="u3", bufs=2)
                u4 = work.tile([B, 64], F32, tag="u4", bufs=2)
                nc.vector.tensor_mul(u1[:], ke, c64)
                nc.vector.tensor_mul(u2[:], ko, s64)
                nc.vector.tensor_sub(k_t[:, 0:KW:2], u1[:], u2[:])
                nc.vector.tensor_mul(u3[:], ke, s64)
                nc.vector.tensor_mul(u4[:], ko, c64)
                nc.vector.tensor_add(k_t[:, 1:KW:2], u3[:], u4[:])

                # v block + ones column
                nc.scalar.copy(vaug_sb[:, si * 129:si * 129 + 128], ps[:, 640:768])
                nc.vector.memset(vaug_sb[:, si * 129 + 128:si * 129 + 129], 1.0)

                # transposes: q (4 blocks) and k (1 block)
                for h in range(HPC):
                    tp = psum.tile([B, B], BF16, tag="C", bufs=2)
                    nc.tensor.transpose(tp[:], q_t[:, h * B:(h + 1) * B], ident_sb[:])
                    dst = qT_sb[:, h * S + si * B:h * S + (si + 1) * B]
                    if h % 2 == 0:
                        nc.vector.tensor_copy(dst, tp[:])
                    else:
                        nc.scalar.copy(dst, tp[:])
                ktp = psum.tile([B, B], BF16, tag="C", bufs=2)
                nc.tensor.transpose(ktp[:], k_t[:], ident_sb[:])
                nc.vector.tensor_copy(kT_sb[:, si * B:(si + 1) * B], ktp[:])

                if si == 7:
                    for h in range(HPC):
                        pT = roam.tile([B, PT_COLS], BF16, tag="roam", bufs=4,
                                       name=f"pT{h}")
                        pTs.append(pT)
                for (h, kj, s1) in sched[si]:
                    _scores(h, kj, s1, chunked=True)
                    done.add((h, kj))
                qi_sched = {12: (0, 4), 13: (1, 5, 8), 14: (2, 6, 9),
                            15: (3, 7, 10, 11)}
                for qi in qi_sched.get(si, ()):
                    _chain_qi(qi)

            # wo reuses the wqkv slot; attnT is its own tensor so chains can
            # write it before the last qkv matmul retires
            big2 = bigp.tile([B, NCB * W768], BF16, tag="big")
            wo_sb = big2[:, 0:HPC * DIM]
            for db in range(HPC):
                nc.sync.dma_start(
                    out=wo_sb[:, db * DIM:(db + 1) * DIM],
                    in_=wo_h[db * B:(db + 1) * B, :],
                )

            _B1_TODO = [
                (h, kj, s1)
                for s0, s1 in SEQ_BLOCKS
                for kj in range(s0, s1)
                for h in range(HPC)
                if (h, kj) not in done
            ]

            # ===== Phase B2: PV + normalize, interleaved with wo per scol =====
            # chains run one scol-group ahead of the output projection so the
            # dense wo matmuls overlap the latency-bound softmax chains
            # seq0/seq1 chains already ran inside phase A; seq2 scores and
            # its chains overlap the dense wo passes
            # seq2: emit each kj's scores then immediately its qi=kj chain
            # (qi needs only exps kj'<=qi), all overlapping wo(0)'s dense work
            _wo(0, use_a=None)
            for kj in range(12, 16):
                for h in range(HPC):
                    if (h, kj) not in done:
                        _scores(h, kj, 16)
                _chain_qi(kj, tp_tag="B", copy_act=True)
            _wo(1, use_a=True)
            _wo(2, use_a=True)
            _wo(3, use_a=True)

    nc.finalize()
    return nc


def get_program():
    global _PROGRAM
    if _PROGRAM is None:
        _PROGRAM = _build_program()
    return _PROGRAM


def make_in_maps(x, cos, sin, wq, wk, wv, wo):
    bf16 = np.dtype("bfloat16") if hasattr(np, "bfloat16") else None
    import ml_dtypes
    bf16 = ml_dtypes.bfloat16

    x = np.asarray(x, np.float32)
    cos = np.asarray(cos, np.float32)
    sin = np.asarray(sin, np.float32)
    wq = np.asarray(wq, np.float32)
    wk = np.asarray(wk, np.float32)
    wv = np.asarray(wv, np.float32)
    wo = np.asarray(wo, np.float32)

    # xt[si, p, cb*B + s] = x[si*B + s, cb*B + p]
    xt = np.ascontiguousarray(
        x.reshape(NSB, B, NCB, B).transpose(0, 3, 2, 1).reshape(NSB, B, DIM)
    ).astype(bf16)
    # cos/sin tiled 4x along channels (per-head repeat), blocked by si
    cosr = np.ascontiguousarray(np.tile(cos, (1, HPC)).reshape(NSB, B, 2 * HEAD_DIM))
    sinr = np.ascontiguousarray(np.tile(sin, (1, HPC)).reshape(NSB, B, 2 * HEAD_DIM))
    # diagonal-block causal bias in scoresT layout: allow sq >= sk
    i = np.arange(B)
    dmask = np.where(i[None, :] >= i[:, None], 0.0, -30000.0).astype(np.float32)
    ident = np.eye(B, dtype=np.float32).astype(bf16)

    scale = HEAD_DIM ** -0.5
    in_maps = []
    for c in range(N_CORES):
        wq_c = (wq[:, c * QW:(c + 1) * QW] * scale).astype(bf16)
        wk_c = wk[:, c * KW:(c + 1) * KW].astype(bf16)
        wv_c = wv[:, c * KW:(c + 1) * KW].astype(bf16)
        wqkv_c = np.ascontiguousarray(
            np.concatenate([wq_c, wk_c, wv_c], axis=1)
        )
        wo_c = np.ascontiguousarray(wo[c * QW:(c + 1) * QW, :]).astype(bf16)
        in_maps.append({
            "xt": xt,
            "wqkv": wqkv_c,
            "wo": wo_c,
            "cosr": cosr,
            "sinr": sinr,
            "dmask": dmask,
            "ident": ident,
        })
    return in_maps


def combine_outputs(results):
    acc = np.zeros((DIM, S), np.float32)
    for r in results:
        acc += np.asarray(r["outp"]).astype(np.float32)
    return np.ascontiguousarray(acc.T)


def kernel(x, cos, sin, mask, wq, wk, wv, wo):
    nc = get_program()
    in_maps = make_in_maps(x, cos, sin, wq, wk, wv, wo)
    res = run_bass_kernel_spmd(nc, in_maps, core_ids=list(range(N_CORES)))
    return combine_outputs(res.results)



# revision 4
# speedup vs baseline: 1.1041x; 1.1041x over previous
"""GQA sparse attention (packed seqs + sliding window + RoPE) on 8 Trainium2 cores.

Sharding: tensor-parallel over heads. Each of the 8 cores owns 4 Q-heads and
their single shared KV-head (GQA groups stay intact): wq columns
[h*512:(h+1)*512], wk/wv columns [h*128:(h+1)*128], wo rows [h*512:(h+1)*512].
Every core computes a full [S, DIM] partial of the output projection; the host
sums the 8 partials.

The mask never reaches the device: seqlens [1024, 512, 512] with causal +
sliding-window 1024 reduce to block-causal over 128-blocks within each
sequence (the window can never truncate since max causal span == 1024), plus
a causal bias on the diagonal 128x128 blocks.

The two dense projections (qkv, wo) run as fp8-e4m3 DoubleRow matmuls
(2 contraction rows per partition, 0.5 PE cycles per output column = 4x bf16
throughput) with a 3-term residual split for accuracy:
    A @ W  ~=  A_hi @ W_hi + A_lo @ W_hi + A_hi @ W_lo
where T_hi = fp8(T), T_lo = fp8(T - T_hi). Weight scales are arranged on the
host so every fp8 tensor sits in e4m3's sweet range: wq/wk/wv carry
512*sqrt(scale), rope's cos/sin carry 1/512 (leaving q,k scaled by
sqrt(scale) so scores come out exact), v's 512*sqrt(scale) cancels against
the softmax denominator by setting the appended ones-column to that same
constant, and wo carries 32 which the host divides back out.

Per-core dataflow (attention middle stays bf16 with fp32 PSUM):
  qkv:   psum[s,768] = sum_t sum_term x?[2t:2t+2].T @ w?[2t:2t+2]   (DR fp8)
  rope:  strided DVE ops on the psum, [s,d] layout, fp32 in / bf16 out
  qT/kT: PE transposes of the roped blocks
  scores(T): psum[sk, sq_span] = kT_blk.T @ qT[h]         (block-causal spans)
  p:     exp(scores + diag_bias) -> pT buffer, bf16
  pv:    psum[sq, 129] = sum_kj pT_blk.T @ [v_blk | ones*F]
  out:   attn = pv[:, :128] * recip(pv[:, 128]); transpose; split fp8 hi/lo
  wo:    psum[c',s] = sum_t sum_term wo?[2t:2t+2].T @ attnT?[2t:2t+2] (DR fp8)
"""

import os

os.environ.setdefault("JAX_PLATFORMS", "axon")

import numpy as np

import concourse.bass as bass
import concourse.mybir as mybir
import concourse.tile as tile
from concourse import bacc
from concourse.bass_utils import run_bass_kernel_spmd

# ---- problem constants (hardcoded per harness contract) ----
DIM = 4096
N_HEADS = 32
N_KV_HEADS = 8
HEAD_DIM = 128
SEQLENS = [1024, 512, 512]
S = 2048
N_CORES = 8
HPC = N_HEADS // N_CORES          # q heads per core = 4
QW = HPC * HEAD_DIM               # per-core q width = 512
KW = HEAD_DIM                     # per-core k/v width = 128
B = 128                           # block size
NSB = S // B                      # 16 seq blocks
NCB = DIM // B                    # 32 contraction blocks
SEQ_BLOCKS = []                   # [(start_blk, end_blk)] per packed sequence
_b = 0
for _l in SEQLENS:
    SEQ_BLOCKS.append((_b, _b + _l // B))
    _b += _l // B

# fp8 scale plumbing (see module docstring)
F_W = 512.0 * (HEAD_DIM ** -0.25)   # wq/wk/wv host scale; ones column value
E_WO = 32.0                         # wo host scale, divided out on host

# pT buffer layout: for each kj, columns [offs[kj] : offs[kj]+span(kj)) hold
# p.T for queries sq in [kj*B, seq_end)
_SPANS = {}
_OFFS = {}
_off = 0
for _s0, _s1 in SEQ_BLOCKS:
    for _kj in range(_s0, _s1):
        _SPANS[_kj] = (_s1 - _kj) * B
        _OFFS[_kj] = _off
        _off += _SPANS[_kj]
PT_COLS = _off                    # 7168

F32 = mybir.dt.float32
BF16 = mybir.dt.bfloat16
F8 = mybir.dt.float8e4
DR = mybir.MatmulPerfMode.DoubleRow

_PROGRAM = None


def _build_program():
    nc = bacc.Bacc(trn_type="TRN2")

    xh_h = nc.declare_dram_parameter("xh", [NSB, B, NCB, B], mybir.dt.uint8, isOutput=False)
    xl_h = nc.declare_dram_parameter("xl", [NSB, B, NCB, B], mybir.dt.uint8, isOutput=False)
    whb_h = nc.declare_dram_parameter("whb", [NCB // 2, B, 2, QW + 2 * KW], mybir.dt.uint8, isOutput=False)
    wlb_h = nc.declare_dram_parameter("wlb", [NCB // 2, B, 2, QW + 2 * KW], mybir.dt.uint8, isOutput=False)
    woh_h = nc.declare_dram_parameter("woh", [HPC // 2, B, 2, DIM], mybir.dt.uint8, isOutput=False)
    wol_h = nc.declare_dram_parameter("wol", [HPC // 2, B, 2, DIM], mybir.dt.uint8, isOutput=False)
    cos_h = nc.declare_dram_parameter("cosr", [NSB, B, 2 * HEAD_DIM], F32, isOutput=False)
    sin_h = nc.declare_dram_parameter("sinr", [NSB, B, 2 * HEAD_DIM], F32, isOutput=False)
    dmask_h = nc.declare_dram_parameter("dmask", [B, B], F32, isOutput=False)
    ident_h = nc.declare_dram_parameter("ident", [B, B], BF16, isOutput=False)
    out_h = nc.declare_dram_parameter("outp", [DIM, S], BF16, isOutput=True)

    W768 = QW + 2 * KW  # 768
    Exp = mybir.ActivationFunctionType.Exp

    with tile.TileContext(nc) as tc:
        with (
            tc.tile_pool(name="consts", bufs=1) as cpool,
            tc.tile_pool(name="big", bufs=1) as bigp,
            tc.tile_pool(name="persist", bufs=1) as pers,
            tc.tile_pool(name="roam", bufs=4) as roam,
            tc.tile_pool(name="work", bufs=3) as work,
            tc.tile_pool(name="psum", bufs=2, space="PSUM") as psum,
        ):
            # first x block before the big weight DMAs so PE starts early
            xh0_t = work.tile([B, NCB, B], F8, tag="xh", bufs=3)
            nc.sync.dma_start(out=xh0_t[:], in_=xh_h[0].bitcast(F8))
            xl0_t = work.tile([B, NCB, B], F8, tag="xl", bufs=3)
            nc.sync.dma_start(out=xl0_t[:], in_=xl_h[0].bitcast(F8))

            # ---- resident tensors ----
            # interleave early x blocks into the weight stream so si=1..3
            # can start before the full wqkv has landed
            early_xh = {}
            early_xl = {}
            wh_sb = bigp.tile([B, NCB, W768], F8, tag="bigh")
            wl_sb = bigp.tile([B, NCB, W768], F8, tag="bigl")
            for t in range(NCB // 2):
                nc.sync.dma_start(out=wh_sb[:, 2 * t:2 * t + 2, :], in_=whb_h[t].bitcast(F8))
                nc.sync.dma_start(out=wl_sb[:, 2 * t:2 * t + 2, :], in_=wlb_h[t].bitcast(F8))
                if t in (3, 8, 12):
                    si_pre = {3: 1, 8: 2, 12: 3}[t]
                    th = work.tile([B, NCB, B], F8, tag="xh", bufs=3, name=f"xhp{si_pre}")
                    nc.sync.dma_start(out=th[:], in_=xh_h[si_pre].bitcast(F8))
                    early_xh[si_pre] = th
                if t in (5, 10, 14):
                    si_pre = {5: 1, 10: 2, 14: 3}[t]
                    tl = work.tile([B, NCB, B], F8, tag="xl", bufs=3, name=f"xlp{si_pre}")
                    nc.sync.dma_start(out=tl[:], in_=xl_h[si_pre].bitcast(F8))
                    early_xl[si_pre] = tl

            # ---- constants (after weights: nothing needs them until rope) ----
            ident_sb = cpool.tile([B, B], BF16)
            nc.sync.dma_start(out=ident_sb[:], in_=ident_h[:])
            dmask_sb = cpool.tile([B, B], F32)
            nc.sync.dma_start(out=dmask_sb[:], in_=dmask_h[:])

            attnT_hi = pers.tile([B, HPC, S], F8)     # [d, head, seq]
            attnT_lo = pers.tile([B, HPC, S], F8)
            qT_sb = pers.tile([B, HPC * S], BF16)     # per head h: cols [h*S, (h+1)*S)
            kT_sb = pers.tile([B, S], BF16)
            vaug_sb = pers.tile([B, NSB * 129], BF16)  # per kj: [v_blk | ones*F_W]

            # scores + exp for one (head, kj) block-row
            pTs = []

            def _scores(h, kj, s1, chunked=False):
                pT = pTs[h]
                span = (s1 - kj) * B
                if chunked:
                    # phase-A-overlapped variant: 512-col chunks through tag B
                    # (PV's tag, idle during phase A) so the qkv psum pipeline
                    # in tag A is never paced by exp latency
                    for part in range(0, span, 512):
                        n = min(512, span - part)
                        ps_c = psum.tile([B, 512], F32, tag="B", bufs=2,
                                         name="ps_c")
                        nc.tensor.matmul(
                            ps_c[:, 0:n],
                            kT_sb[:, kj * B:(kj + 1) * B],
                            qT_sb[:, h * S + kj * B + part:
                                  h * S + kj * B + part + n],
                            start=True, stop=True,
                        )
                        if part == 0:
                            nc.vector.tensor_add(
                                ps_c[:, 0:B], ps_c[:, 0:B], dmask_sb[:]
                            )
                        nc.scalar.activation(
                            pT[:, _OFFS[kj] + part:_OFFS[kj] + part + n],
                            ps_c[:, 0:n], Exp
                        )
                    return
                ps_sc = psum.tile([B, 1024], F32, tag="A", bufs=2, name="ps_sc")
                for part in range(0, span, 512):
                    n = min(512, span - part)
                    nc.tensor.matmul(
                        ps_sc[:, part:part + n],
                        kT_sb[:, kj * B:(kj + 1) * B],
                        qT_sb[:, h * S + kj * B + part:
                              h * S + kj * B + part + n],
                        start=True, stop=True,
                    )
                # causal bias on the diagonal block
                nc.vector.tensor_add(ps_sc[:, 0:B], ps_sc[:, 0:B], dmask_sb[:])
                nc.scalar.activation(
                    pT[:, _OFFS[kj]:_OFFS[kj] + span], ps_sc[:, 0:span], Exp
                )

            # seq0/seq1 score groups overlap the back half of the qkv phase
            # (their qT/kT inputs are complete by then); seq2 runs after
            chains_done = set()
            sched = {si: [] for si in range(NSB)}
            for kj in range(0, 8):
                sched[8 + kj] = [(h, kj, 8) for h in range(HPC)]
            for kj, si in ((8, 12), (9, 13), (10, 14), (11, 15)):
                sched[si] += [(h, kj, 12) for h in range(HPC)]
            done = set()

            def _chain_qi(qi, tp_tag="C"):
                    chains_done.add(qi)
                    s0, s1 = next(b for b in SEQ_BLOCKS if b[0] <= qi < b[1])
                    for h in range(HPC):
                        pT = pTs[h]
                        ps_pv = psum.tile([B, 129], F32, tag="B", bufs=2)
                        for kj in range(s0, qi + 1):
                            lhsT = pT[:, _OFFS[kj] + (qi - kj) * B:
                                      _OFFS[kj] + (qi - kj + 1) * B]
                            nc.tensor.matmul(
                                ps_pv[:], lhsT,
                                vaug_sb[:, kj * 129:(kj + 1) * 129],
                                start=(kj == s0), stop=(kj == qi),
                            )
                        pv_sb = work.tile([B, 129], F32, tag="pv", bufs=8)
                        nc.vector.tensor_copy(pv_sb[:], ps_pv[:])
                        rc = work.tile([B, 1], F32, tag="rc", bufs=8)
                        nc.vector.reciprocal(rc[:], pv_sb[:, 128:129])
                        at = work.tile([B, B], BF16, tag="at", bufs=8)
                        nc.vector.tensor_scalar_mul(at[:], pv_sb[:, 0:B], rc[:])
                        tp = psum.tile([B, B], BF16, tag=tp_tag, bufs=2)
                        nc.tensor.transpose(tp[:], at[:], ident_sb[:])
                        hi_dst = attnT_hi[:, h, qi * B:(qi + 1) * B]
                        lo_dst = attnT_lo[:, h, qi * B:(qi + 1) * B]
                        nc.scalar.copy(hi_dst, tp[:])
                        nc.vector.tensor_sub(lo_dst, tp[:], hi_dst)

            def _wo(scol, use_a=False):
                for cp in range(NCB):
                    for sub in range(2):
                        m = cp * 2 + sub
                        # alternate accumulators across tags (A idles in the
                        # late passes; B drains after seq2 chains)
                        if use_a and m % 2 == 1:
                            psoF = psum.tile([B, 512], F32, tag="A", bufs=2,
                                             name="psoA")
                        elif use_a is None and m >= 32 and m % 2 == 1:
                            psoF = psum.tile([B, 512], F32, tag="B", bufs=2,
                                             name="psoB")
                        else:
                            psoF = psum.tile([B, 512], F32, tag="C", bufs=2,
                                             name="pso")
                        pso = psoF[:, 0:256]
                        c0 = scol * 512 + sub * 256
                        nmm = 0
                        for t in range(HPC // 2):
                            for wa, aa in ((woh_sb, attnT_hi),
                                           (wol_sb, attnT_hi),
                                           (woh_sb, attnT_lo)):
                                nc.tensor.matmul(
                                    pso,
                                    wa[:, 2 * t:2 * t + 2, cp * B:(cp + 1) * B],
                                    aa[:, 2 * t:2 * t + 2, c0:c0 + 256],
                                    start=(nmm == 0), stop=(nmm == 5),
                                    perf_mode=DR,
                                )
                                nmm += 1
                        if sub == 0:
                            ot = work.tile([B, 512], BF16, tag="ot", bufs=8)
                        if m % 2 == 0:
                            nc.scalar.copy(ot[:, sub * 256:(sub + 1) * 256], pso)
                        else:
                            nc.vector.tensor_copy(ot[:, sub * 256:(sub + 1) * 256], pso)
                        if sub == 1:
                            nc.sync.dma_start(
                                out=out_h[cp * B:(cp + 1) * B,
                                          scol * 512:(scol + 1) * 512],
                                in_=ot[:],
                            )

            # =========== Phase A: qkv projection + rope + transposes ===========
            for si in range(NSB):
                if si == 0:
                    xh_t, xl_t = xh0_t, xl0_t
                elif si in early_xh:
                    xh_t, xl_t = early_xh[si], early_xl[si]
                else:
                    xh_t = work.tile([B, NCB, B], F8, tag="xh", bufs=3)
                    nc.sync.dma_start(out=xh_t[:], in_=xh_h[si].bitcast(F8))
                    xl_t = work.tile([B, NCB, B], F8, tag="xl", bufs=3)
                    nc.sync.dma_start(out=xl_t[:], in_=xl_h[si].bitcast(F8))
                # small rotating cos/sin tiles (dead after rope of this si)
                c_t = work.tile([B, 256], F32, tag="cs", bufs=3)
                nc.sync.dma_start(out=c_t[:], in_=cos_h[si])
                s_t = work.tile([B, 256], F32, tag="sn", bufs=3)
                nc.sync.dma_start(out=s_t[:], in_=sin_h[si])
                ps = psum.tile([B, W768], F32, tag="A", bufs=2)
                # kv chunk first so rope-k/v copy unblock earliest
                for c0, c1 in ((512, 768), (0, 256), (256, 512)):
                    nmm = 0
                    last = 3 * (NCB // 2) - 1
                    for t in range(NCB // 2):
                        for xa, wa in ((xh_t, wh_sb), (xl_t, wh_sb), (xh_t, wl_sb)):
                            nc.tensor.matmul(
                                ps[:, c0:c1],
                                xa[:, 2 * t:2 * t + 2, :],
                                wa[:, 2 * t:2 * t + 2, c0:c1],
                                start=(nmm == 0), stop=(nmm == last),
                                perf_mode=DR,
                            )
                            nmm += 1

                cs = c_t[:]
                sn = s_t[:]

                # rope on k first (kT feeds the phase-A score groups)
                k_t = work.tile([B, KW], BF16, tag="k", bufs=3)
                ke, ko = ps[:, 512:640:2], ps[:, 513:640:2]
                c64, s64 = c_t[:, 0:64], s_t[:, 0:64]
                u1 = work.tile([B, 64], F32, tag="u1", bufs=2)
                u2 = work.tile([B, 64], F32, tag="u2", bufs=2)
                u3 = work.tile([B, 64], F32, tag="u3", bufs=2)
                u4 = work.tile([B, 64], F32, tag="u4", bufs=2)
                nc.vector.tensor_mul(u1[:], ke, c64)
                nc.vector.tensor_mul(u2[:], ko, s64)
                nc.vector.tensor_sub(k_t[:, 0:KW:2], u1[:], u2[:])
                nc.vector.tensor_mul(u3[:], ke, s64)
                nc.vector.tensor_mul(u4[:], ko, c64)
                nc.vector.tensor_add(k_t[:, 1:KW:2], u3[:], u4[:])

                # v block + ones column (ones = F_W cancels v's host scale)
                nc.scalar.copy(vaug_sb[:, si * 129:si * 129 + 128], ps[:, 640:768])
                nc.vector.memset(vaug_sb[:, si * 129 + 128:si * 129 + 129], F_W)

                # rope on q: [s, d] layout, channels interleaved (even, odd)
                q_t = work.tile([B, QW], BF16, tag="q", bufs=3)
                qe, qo = ps[:, 0:QW:2], ps[:, 1:QW:2]
                t1 = work.tile([B, 256], F32, tag="t1", bufs=2)
                t2 = work.tile([B, 256], F32, tag="t2", bufs=2)
                t3 = work.tile([B, 256], F32, tag="t3", bufs=2)
                t4 = work.tile([B, 256], F32, tag="t4", bufs=2)
                nc.vector.tensor_mul(t1[:], qe, cs)
                nc.vector.tensor_mul(t2[:], qo, sn)
                nc.vector.tensor_sub(q_t[:, 0:QW:2], t1[:], t2[:])
                nc.vector.tensor_mul(t3[:], qe, sn)
                nc.vector.tensor_mul(t4[:], qo, cs)
                nc.vector.tensor_add(q_t[:, 1:QW:2], t3[:], t4[:])

                # transposes: k first (feeds scores), then q (4 blocks)
                ktp = psum.tile([B, B], BF16, tag="C", bufs=2)
                nc.tensor.transpose(ktp[:], k_t[:], ident_sb[:])
                nc.vector.tensor_copy(kT_sb[:, si * B:(si + 1) * B], ktp[:])
                for h in range(HPC):
                    tp = psum.tile([B, B], BF16, tag="C", bufs=2)
                    nc.tensor.transpose(tp[:], q_t[:, h * B:(h + 1) * B], ident_sb[:])
                    dst = qT_sb[:, h * S + si * B:h * S + (si + 1) * B]
                    if h % 2 == 0:
                        nc.vector.tensor_copy(dst, tp[:])
                    else:
                        nc.scalar.copy(dst, tp[:])

                if si == 7:
                    for h in range(HPC):
                        pT = roam.tile([B, PT_COLS], BF16, tag="roam", bufs=4,
                                       name=f"pT{h}")
                        pTs.append(pT)
                for (h, kj, s1) in sched[si]:
                    _scores(h, kj, s1, chunked=True)
                    done.add((h, kj))
                qi_sched = {12: (0, 4), 13: (1, 5, 8), 14: (2, 6, 9),
                            15: (3, 7, 10, 11)}
                for qi in qi_sched.get(si, ()):
                    _chain_qi(qi)

            # wo reuses the wqkv slots; attnT is its own tensor so chains can
            # write it before the last qkv matmul retires
            woh_sb = bigp.tile([B, HPC, DIM], F8, tag="bigh")
            wol_sb = bigp.tile([B, HPC, DIM], F8, tag="bigl")
            for t in range(HPC // 2):
                nc.sync.dma_start(out=woh_sb[:, 2 * t:2 * t + 2, :], in_=woh_h[t].bitcast(F8))
                nc.sync.dma_start(out=wol_sb[:, 2 * t:2 * t + 2, :], in_=wol_h[t].bitcast(F8))

            # ===== Phase B2: PV + normalize, interleaved with wo per scol =====
            # seq0/seq1 chains already ran inside phase A; seq2 scores and
            # its chains overlap the dense wo passes
            _wo(0, use_a=None)
            for kj in range(12, 16):
                for h in range(HPC):
                    if (h, kj) not in done:
                        _scores(h, kj, 16)
                _chain_qi(kj, tp_tag="B")
            _wo(1, use_a=True)
            _wo(2, use_a=True)
            _wo(3, use_a=True)

    nc.finalize()
    return nc


def get_program():
    global _PROGRAM
    if _PROGRAM is None:
        _PROGRAM = _build_program()
    return _PROGRAM


def make_in_maps(x, cos, sin, wq, wk, wv, wo):
    import ml_dtypes
    f8 = ml_dtypes.float8_e4m3fn

    def split8(v):
        hi = v.astype(f8)
        lo = (v - hi.astype(np.float32)).astype(f8)
        return hi, lo

    x = np.asarray(x, np.float32)
    cos = np.asarray(cos, np.float32)
    sin = np.asarray(sin, np.float32)
    wq = np.asarray(wq, np.float32)
    wk = np.asarray(wk, np.float32)
    wv = np.asarray(wv, np.float32)
    wo = np.asarray(wo, np.float32)

    # xt[si, p, cb, s] = x[si*B + s, cb*B + p]
    xt = np.ascontiguousarray(
        x.reshape(NSB, B, NCB, B).transpose(0, 3, 2, 1)
    )
    xh, xl = split8(xt)
    # cos/sin tiled 4x along channels (per-head repeat), blocked by si,
    # divided by 512 to cancel the 512*sqrt(scale) on wq/wk (leaving sqrt(scale))
    cosr = np.ascontiguousarray(
        np.tile(cos / 512.0, (1, HPC)).reshape(NSB, B, 2 * HEAD_DIM))
    sinr = np.ascontiguousarray(
        np.tile(sin / 512.0, (1, HPC)).reshape(NSB, B, 2 * HEAD_DIM))
    # diagonal-block causal bias in scoresT layout: allow sq >= sk
    i = np.arange(B)
    dmask = np.where(i[None, :] >= i[:, None], 0.0, -30000.0).astype(np.float32)
    ident = np.eye(B, dtype=np.float32).astype(ml_dtypes.bfloat16)

    W768 = QW + 2 * KW
    in_maps = []
    for c in range(N_CORES):
        wq_c = wq[:, c * QW:(c + 1) * QW] * F_W
        wk_c = wk[:, c * KW:(c + 1) * KW] * F_W
        wv_c = wv[:, c * KW:(c + 1) * KW] * F_W
        wqkv_c = np.concatenate([wq_c, wk_c, wv_c], axis=1)
        # blocked [t, p, i, j] = wqkv[(2t+i)*B + p, j]
        wqkv_b = np.ascontiguousarray(
            wqkv_c.reshape(NCB // 2, 2, B, W768).transpose(0, 2, 1, 3))
        whb, wlb = split8(wqkv_b)
        wo_c = wo[c * QW:(c + 1) * QW, :] * E_WO
        wo_b = np.ascontiguousarray(
            wo_c.reshape(HPC // 2, 2, B, DIM).transpose(0, 2, 1, 3))
        woh, wol = split8(wo_b)
        in_maps.append({
            "xh": xh.view(np.uint8),
            "xl": xl.view(np.uint8),
            "whb": whb.view(np.uint8),
            "wlb": wlb.view(np.uint8),
            "woh": woh.view(np.uint8),
            "wol": wol.view(np.uint8),
            "cosr": cosr,
            "sinr": sinr,
            "dmask": dmask,
            "ident": ident,
        })
    return in_maps


def combine_outputs(results):
    acc = np.zeros((DIM, S), np.float32)
    for r in results:
        acc += np.asarray(r["outp"]).astype(np.float32)
    acc *= 1.0 / E_WO
    return np.ascontiguousarray(acc.T)


def kernel(x, cos, sin, mask, wq, wk, wv, wo):
    nc = get_program()
    in_maps = make_in_maps(x, cos, sin, wq, wk, wv, wo)
    res = run_bass_kernel_spmd(nc, in_maps, core_ids=list(range(N_CORES)))
    return combine_outputs(res.results)


# revision 20
# speedup vs baseline: 1.1819x; 1.0705x over previous
"""GQA sparse attention (packed seqs + sliding window + RoPE) on 8 Trainium2 cores.

Sharding: tensor-parallel over heads. Each of the 8 cores owns 4 Q-heads and
their single shared KV-head (GQA groups stay intact): wq columns
[h*512:(h+1)*512], wk/wv columns [h*128:(h+1)*128], wo rows [h*512:(h+1)*512].
Every core computes a full [S, DIM] partial of the output projection; the host
sums the 8 partials.

The mask never reaches the device: seqlens [1024, 512, 512] with causal +
sliding-window 1024 reduce to block-causal over 128-blocks within each
sequence (the window can never truncate since max causal span == 1024), plus
a causal bias on the diagonal 128x128 blocks.

The two dense projections (qkv, wo) run as fp8-e4m3 DoubleRow matmuls
(2 contraction rows per partition, 0.5 PE cycles per output column = 4x bf16
throughput) with a 3-term residual split for accuracy:
    A @ W  ~=  A_hi @ W_hi + A_lo @ W_hi + A_hi @ W_lo
where T_hi = fp8(T), T_lo = fp8(T - T_hi). Weight scales are arranged on the
host so every fp8 tensor sits in e4m3's sweet range: wq/wk/wv carry
512*sqrt(scale), rope's cos/sin carry 1/512 (leaving q,k scaled by
sqrt(scale) so scores come out exact), v's 512*sqrt(scale) cancels against
the softmax denominator by setting the appended ones-column to that same
constant, and wo carries 32 which the host divides back out.

Per-core dataflow (attention middle stays bf16 with fp32 PSUM):
  qkv:   psum[s,768] = sum_t sum_term x?[2t:2t+2].T @ w?[2t:2t+2]   (DR fp8)
  rope:  strided DVE ops on the psum, [s,d] layout, fp32 in / bf16 out
  qT/kT: PE transposes of the roped blocks
  scores(T): psum[sk, sq_span] = kT_blk.T @ qT[h]         (block-causal spans)
  p:     exp(scores + diag_bias) -> pT buffer, bf16
  pv:    psum[sq, 129] = sum_kj pT_blk.T @ [v_blk | ones*F]
  out:   attn = pv[:, :128] * recip(pv[:, 128]); transpose; split fp8 hi/lo
  wo:    psum[c',s] = sum_t sum_term wo?[2t:2t+2].T @ attnT?[2t:2t+2] (DR fp8)
"""

import os

os.environ.setdefault("JAX_PLATFORMS", "axon")

import numpy as np

import concourse.bass as bass
import concourse.mybir as mybir
import concourse.tile as tile
from concourse import bacc
from concourse.bass_utils import run_bass_kernel_spmd

# ---- problem constants (hardcoded per harness contract) ----
DIM = 4096
N_HEADS = 32
N_KV_HEADS = 8
HEAD_DIM = 128
SEQLENS = [1024, 512, 512]
S = 2048
N_CORES = 8
HPC = N_HEADS // N_CORES          # q heads per core = 4
QW = HPC * HEAD_DIM               # per-core q width = 512
KW = HEAD_DIM                     # per-core k/v width = 128
B = 128                           # block size
NSB = S // B                      # 16 seq blocks
NCB = DIM // B                    # 32 contraction blocks
SEQ_BLOCKS = []                   # [(start_blk, end_blk)] per packed sequence
_b = 0
for _l in SEQLENS:
    SEQ_BLOCKS.append((_b, _b + _l // B))
    _b += _l // B

# fp8 scale plumbing (see module docstring)
F_W = 512.0 * (HEAD_DIM ** -0.25)   # wq/wk/wv host scale; ones column value
E_WO = 32.0                         # wo host scale, divided out on host

# pT buffer layout: for each kj, columns [offs[kj] : offs[kj]+span(kj)) hold
# p.T for queries sq in [kj*B, seq_end)
_SPANS = {}
_OFFS = {}
_off = 0
for _s0, _s1 in SEQ_BLOCKS:
    for _kj in range(_s0, _s1):
        _SPANS[_kj] = (_s1 - _kj) * B
        _OFFS[_kj] = _off
        _off += _SPANS[_kj]
PT_COLS = _off                    # 7168

F32 = mybir.dt.float32
BF16 = mybir.dt.bfloat16
F8 = mybir.dt.float8e4
DR = mybir.MatmulPerfMode.DoubleRow

_PROGRAM = None


def _build_program():
    nc = bacc.Bacc(trn_type="TRN2")

    xh_h = nc.declare_dram_parameter("xh", [NSB, B, NCB, B], mybir.dt.uint8, isOutput=False)
    xl_h = nc.declare_dram_parameter("xl", [NSB, B, NCB, B], mybir.dt.uint8, isOutput=False)
    whb_h = nc.declare_dram_parameter("whb", [NCB // 8, B, 8, QW + 2 * KW], mybir.dt.uint8, isOutput=False)
    wlb_h = nc.declare_dram_parameter("wlb", [NCB // 8, B, 8, QW + 2 * KW], mybir.dt.uint8, isOutput=False)
    woh_h = nc.declare_dram_parameter("woh", [HPC // 2, B, 2, DIM], mybir.dt.uint8, isOutput=False)
    wol_h = nc.declare_dram_parameter("wol", [HPC // 2, B, 2, DIM], mybir.dt.uint8, isOutput=False)
    cs_h = nc.declare_dram_parameter("csr", [NSB, B, 4 * HEAD_DIM], F32, isOutput=False)
    dmask_h = nc.declare_dram_parameter("dmask", [B, B], F32, isOutput=False)
    ident_h = nc.declare_dram_parameter("ident", [B, B], BF16, isOutput=False)
    # out layout [cpg, p, j, scol, col]: a [B, 4, 512] SBUF tile DMAs to
    # outp[cpg, :, :, scol, :] with matching iteration order (host reassembles)
    out_h = nc.declare_dram_parameter("outp", [NCB // 4, B, 4, 4, 512], BF16, isOutput=True)

    W768 = QW + 2 * KW  # 768
    Exp = mybir.ActivationFunctionType.Exp

    with tile.TileContext(nc) as tc:
        with (
            tc.tile_pool(name="consts", bufs=1) as cpool,
            tc.tile_pool(name="big", bufs=1) as bigp,
            tc.tile_pool(name="persist", bufs=1) as pers,
            tc.tile_pool(name="roam", bufs=4) as roam,
            tc.tile_pool(name="work", bufs=3) as work,
            tc.tile_pool(name="psum", bufs=2, space="PSUM") as psum,
        ):
            # startup is DMA-bound: the stream order below is tuned so PE
            # always has runnable qkv work (see EARLY_DMA / EARLY_PE)
            wh_sb = bigp.tile([B, NCB, W768], F8, tag="bigh")
            wl_sb = bigp.tile([B, NCB, W768], F8, tag="bigl")
            early_xh = {}
            early_xl = {}
            early_cs = {}

            EARLY_DMA = [
                ("xhA", 0), ("wh", 0), ("xlA", 0), ("xhB", 0), ("wh", 1),
                ("xlB", 0), ("wh", 2), ("wh", 3), ("xh", 1), ("xl", 1),
                ("const",), ("cs", 0), ("cs", 1), ("wl", 0), ("wl", 1),
                ("wl", 2), ("wl", 3), ("xh", 2), ("xl", 2), ("cs", 2),
                ("xh", 3), ("xl", 3), ("cs", 3),
            ]
            ident_sb = cpool.tile([B, B], BF16)
            dmask_sb = cpool.tile([B, B], F32)
            for ent in EARLY_DMA:
                kind = ent[0]
                if kind in ("xhA", "xlA", "xhB", "xlB"):
                    si_, hl = ent[1], kind[1]
                    dram = xh_h if hl == "h" else xl_h
                    store = early_xh if hl == "h" else early_xl
                    if kind[2] == "A":
                        t_ = work.tile([B, NCB, B], F8, tag="x" + hl, bufs=3,
                                       name=f"x{hl}p{si_}")
                        store[si_] = t_
                        nc.sync.dma_start(out=t_[:, 0:16, :],
                                          in_=dram[si_][:, 0:16, :].bitcast(F8))
                    else:
                        t_ = store[si_]
                        nc.sync.dma_start(out=t_[:, 16:32, :],
                                          in_=dram[si_][:, 16:32, :].bitcast(F8))
                elif kind == "xh":
                    t_ = work.tile([B, NCB, B], F8, tag="xh", bufs=3,
                                   name=f"xhp{ent[1]}")
                    nc.sync.dma_start(out=t_[:], in_=xh_h[ent[1]].bitcast(F8))
                    early_xh[ent[1]] = t_
                elif kind == "xl":
                    t_ = work.tile([B, NCB, B], F8, tag="xl", bufs=3,
                                   name=f"xlp{ent[1]}")
                    nc.sync.dma_start(out=t_[:], in_=xl_h[ent[1]].bitcast(F8))
                    early_xl[ent[1]] = t_
                elif kind == "wh":
                    b = ent[1]
                    nc.sync.dma_start(out=wh_sb[:, 8 * b:8 * b + 8, :],
                                      in_=whb_h[b].bitcast(F8))
                elif kind == "wl":
                    b = ent[1]
                    nc.sync.dma_start(out=wl_sb[:, 8 * b:8 * b + 8, :],
                                      in_=wlb_h[b].bitcast(F8))
                elif kind == "cs":
                    t_ = work.tile([B, 512], F32, tag="cs", bufs=3,
                                   name=f"csp{ent[1]}")
                    nc.sync.dma_start(out=t_[:], in_=cs_h[ent[1]])
                    early_cs[ent[1]] = t_
                else:
                    nc.sync.dma_start(out=ident_sb[:], in_=ident_h[:])
                    nc.sync.dma_start(out=dmask_sb[:], in_=dmask_h[:])

            attnT_hi = pers.tile([B, HPC, S], F8)     # [d, head, seq]
            attnT_lo = pers.tile([B, HPC, S], F8)
            qT_sb = pers.tile([B, HPC * S], BF16)     # per head h: cols [h*S, (h+1)*S)
            kT_sb = pers.tile([B, S], BF16)
            vaug_sb = pers.tile([B, NSB * 129], BF16)  # per kj: [v_blk | ones*F_W]

            # scores + exp for one (head, kj) block-row
            pTs = []

            def _scores(h, kj, s1, chunked=False):
                pT = pTs[h]
                span = (s1 - kj) * B
                if chunked:
                    # phase-A-overlapped variant: 512-col chunks through tag B
                    # (PV's tag, idle during phase A) so the qkv psum pipeline
                    # in tag A is never paced by exp latency
                    for part in range(0, span, 512):
                        n = min(512, span - part)
                        ps_c = psum.tile([B, 512], F32, tag="B", bufs=2,
                                         name="ps_c")
                        nc.tensor.matmul(
                            ps_c[:, 0:n],
                            kT_sb[:, kj * B:(kj + 1) * B],
                            qT_sb[:, h * S + kj * B + part:
                                  h * S + kj * B + part + n],
                            start=True, stop=True,
                        )
                        if part == 0:
                            nc.vector.tensor_add(
                                ps_c[:, 0:B], ps_c[:, 0:B], dmask_sb[:]
                            )
                        nc.scalar.activation(
                            pT[:, _OFFS[kj] + part:_OFFS[kj] + part + n],
                            ps_c[:, 0:n], Exp
                        )
                    return
                ps_sc = psum.tile([B, 1024], F32, tag="A", bufs=2, name="ps_sc")
                for part in range(0, span, 512):
                    n = min(512, span - part)
                    nc.tensor.matmul(
                        ps_sc[:, part:part + n],
                        kT_sb[:, kj * B:(kj + 1) * B],
                        qT_sb[:, h * S + kj * B + part:
                              h * S + kj * B + part + n],
                        start=True, stop=True,
                    )
                # causal bias on the diagonal block
                nc.vector.tensor_add(ps_sc[:, 0:B], ps_sc[:, 0:B], dmask_sb[:])
                nc.scalar.activation(
                    pT[:, _OFFS[kj]:_OFFS[kj] + span], ps_sc[:, 0:span], Exp
                )

            # seq0/seq1 score groups overlap the back half of the qkv phase
            # (their qT/kT inputs are complete by then); seq2 runs after
            chains_done = set()
            sched = {si: [] for si in range(NSB)}
            for kj in range(0, 8):
                sched[8 + kj] = [(h, kj, 8) for h in range(HPC)]
            for kj, si in ((8, 12), (9, 13), (10, 14), (11, 15)):
                sched[si] += [(h, kj, 12) for h in range(HPC)]
            done = set()

            def _chain_head(qi, h, tp_tag="C"):
                    s0, s1 = next(b for b in SEQ_BLOCKS if b[0] <= qi < b[1])
                    if True:
                        pT = pTs[h]
                        ps_pv = psum.tile([B, 129], F32, tag="B", bufs=2)
                        for kj in range(s0, qi + 1):
                            lhsT = pT[:, _OFFS[kj] + (qi - kj) * B:
                                      _OFFS[kj] + (qi - kj + 1) * B]
                            nc.tensor.matmul(
                                ps_pv[:], lhsT,
                                vaug_sb[:, kj * 129:(kj + 1) * 129],
                                start=(kj == s0), stop=(kj == qi),
                            )
                        pv_sb = work.tile([B, 129], F32, tag="pv", bufs=6)
                        nc.vector.tensor_copy(pv_sb[:], ps_pv[:])
                        rc = work.tile([B, 1], F32, tag="rc", bufs=8)
                        nc.vector.reciprocal(rc[:], pv_sb[:, 128:129])
                        at = work.tile([B, B], BF16, tag="at", bufs=6)
                        nc.vector.tensor_scalar_mul(at[:], pv_sb[:, 0:B], rc[:])
                        tp = psum.tile([B, B], BF16, tag=tp_tag, bufs=2)
                        nc.tensor.transpose(tp[:], at[:], ident_sb[:])
                        hi_dst = attnT_hi[:, h, qi * B:(qi + 1) * B]
                        lo_dst = attnT_lo[:, h, qi * B:(qi + 1) * B]
                        nc.scalar.copy(hi_dst, tp[:])
                        nc.vector.tensor_sub(lo_dst, tp[:], hi_dst)

            ot_cur = {}

            def _chain_qi(qi, tp_tag="C"):
                chains_done.add(qi)
                for h in range(HPC):
                    _chain_head(qi, h, tp_tag)

            def _wo(scol, cps=None):
                for cp in (range(NCB) if cps is None else cps):
                    # alternate accumulators across tags C/A (the qkv psum in
                    # A is released by the stage-copy; B stays with chains)
                    if cp % 2 == 1:
                        psoF = psum.tile([B, 512], F32, tag="A", bufs=2,
                                         name="psoA")
                    else:
                        psoF = psum.tile([B, 512], F32, tag="C", bufs=2,
                                         name="psoC")
                    for sub in range(2):
                        pso = psoF[:, sub * 256:(sub + 1) * 256]
                        c0 = scol * 512 + sub * 256
                        nmm = 0
                        for t in range(HPC // 2):
                            for wa, aa in ((woh_sb, attnT_hi),
                                           (wol_sb, attnT_hi),
                                           (woh_sb, attnT_lo)):
                                nc.tensor.matmul(
                                    pso,
                                    wa[:, 2 * t:2 * t + 2, cp * B:(cp + 1) * B],
                                    aa[:, 2 * t:2 * t + 2, c0:c0 + 256],
                                    start=(nmm == 0), stop=(nmm == 5),
                                    perf_mode=DR,
                                )
                                nmm += 1
                    j = cp % 4
                    if scol == 3 and cp >= 28:
                        # final cps: per-cp DMA right after each copy so the
                        # post-PE drain is one copy + one small DMA deep
                        ot1 = work.tile([B, 512], BF16, tag="ot1", bufs=2,
                                        name="ott")
                        if cp % 2 == 0:
                            nc.scalar.copy(ot1[:], psoF[:])
                        else:
                            nc.vector.tensor_copy(ot1[:], psoF[:])
                        nc.sync.dma_start(out=out_h[cp // 4][:, j, scol, :],
                                          in_=ot1[:])
                        continue
                    if j == 0:
                        ot_cur[scol] = work.tile([B, 4, 512], BF16, tag="ot",
                                                 bufs=2, name="otb")
                    ot = ot_cur[scol]
                    if cp % 2 == 0:
                        nc.scalar.copy(ot[:, j, :], psoF[:])
                    else:
                        nc.vector.tensor_copy(ot[:, j, :], psoF[:])
                    if j == 3:
                        nc.sync.dma_start(
                            out=out_h[cp // 4][:, :, scol, :],
                            in_=ot[:],
                        )

            # =========== Phase A: qkv projection + rope + transposes ===========
            # Per si, the three residual terms split into a hi-part (terms
            # needing only the hi weights: xh@wh, xl@wh) and a lo-part
            # (xh@wl); the psum group opens at the first hi matmul and closes
            # at the last lo matmul. For si 0-3 the parts are emitted in a
            # hand-tuned order interleaved with the DMA stream; si 4-15 are
            # PE-bound and run parts back to back.
            ps_by_si = {}
            CHUNKS = ((512, 768), (0, 256), (256, 512))

            def _qkv_part(si, term, b0, b1):
                if si not in ps_by_si:
                    ps_by_si[si] = psum.tile([B, W768], F32, tag="A", bufs=2,
                                             name=f"ps{si % 2}")
                ps = ps_by_si[si]
                xa = early_xh[si] if term != 2 else early_xl[si]
                wa = wh_sb if term != 3 else wl_sb
                # chunks q0 [0:256] and q1 [256:512] share one 2KB psum
                # zero region: only q0's first matmul carries start=True (its
                # pending-zero covers q1's bytes); giving q1 its own start
                # while q0's group is still open would wipe q0's partials
                for c0, c1 in CHUNKS:
                    for t in range(4 * b0, 4 * b1):
                        nc.tensor.matmul(
                            ps[:, c0:c1],
                            xa[:, 2 * t:2 * t + 2, :],
                            wa[:, 2 * t:2 * t + 2, c0:c1],
                            start=(term == 1 and t == 0 and c0 != 256),
                            stop=(term == 3 and t == NCB // 2 - 1),
                            perf_mode=DR,
                            skip_group_check=True,
                        )

            def _finish_si(si):
                psum_ps = ps_by_si.pop(si)
                cs_t = early_cs.pop(si)
                cs = cs_t[:, 0:256]
                sn = cs_t[:, 256:512]

                # stage the qkv psum to SBUF in one fast Act copy: releases
                # the psum buffer for si+2 immediately, and the strided rope
                # reads below hit SBUF (58-cycle access) instead of PSUM (120)
                ps = work.tile([B, W768], F32, tag="qkvs", bufs=2)
                nc.scalar.copy(ps[:], psum_ps[:])

                # rope on k first (kT feeds the phase-A score groups)
                k_t = work.tile([B, KW], BF16, tag="k", bufs=3)
                ke, ko = ps[:, 512:640:2], ps[:, 513:640:2]
                c64, s64 = cs_t[:, 0:64], cs_t[:, 256:320]
                u1 = work.tile([B, 64], F32, tag="u1", bufs=2)
                u2 = work.tile([B, 64], F32, tag="u2", bufs=2)
                u3 = work.tile([B, 64], F32, tag="u1", bufs=2, name="u3")
                u4 = work.tile([B, 64], F32, tag="u2", bufs=2, name="u4")
                nc.vector.tensor_mul(u1[:], ke, c64)
                nc.vector.tensor_mul(u2[:], ko, s64)
                nc.vector.tensor_sub(k_t[:, 0:KW:2], u1[:], u2[:])
                nc.vector.tensor_mul(u3[:], ke, s64)
                nc.vector.tensor_mul(u4[:], ko, c64)
                nc.vector.tensor_add(k_t[:, 1:KW:2], u3[:], u4[:])

                # v block + ones column (ones = F_W cancels v's host scale)
                nc.scalar.copy(vaug_sb[:, si * 129:si * 129 + 128], ps[:, 640:768])
                nc.vector.memset(vaug_sb[:, si * 129 + 128:si * 129 + 129], F_W)

                # rope on q: [s, d] layout, channels interleaved (even, odd)
                q_t = work.tile([B, QW], BF16, tag="q", bufs=3)
                qe, qo = ps[:, 0:QW:2], ps[:, 1:QW:2]
                t1 = work.tile([B, 256], F32, tag="t1", bufs=2)
                t2 = work.tile([B, 256], F32, tag="t2", bufs=2)
                t3 = work.tile([B, 256], F32, tag="t1", bufs=2, name="t3")
                t4 = work.tile([B, 256], F32, tag="t2", bufs=2, name="t4")
                nc.vector.tensor_mul(t1[:], qe, cs)
                nc.vector.tensor_mul(t2[:], qo, sn)
                nc.vector.tensor_sub(q_t[:, 0:QW:2], t1[:], t2[:])
                nc.vector.tensor_mul(t3[:], qe, sn)
                nc.vector.tensor_mul(t4[:], qo, cs)
                nc.vector.tensor_add(q_t[:, 1:QW:2], t3[:], t4[:])

                # transposes: k first (feeds scores), then q (4 blocks)
                ktp = psum.tile([B, B], BF16, tag="C", bufs=2)
                nc.tensor.transpose(ktp[:], k_t[:], ident_sb[:])
                nc.vector.tensor_copy(kT_sb[:, si * B:(si + 1) * B], ktp[:])
                for h in range(HPC):
                    tp = psum.tile([B, B], BF16, tag="C", bufs=2)
                    nc.tensor.transpose(tp[:], q_t[:, h * B:(h + 1) * B], ident_sb[:])
                    dst = qT_sb[:, h * S + si * B:h * S + (si + 1) * B]
                    if h % 2 == 0:
                        nc.vector.tensor_copy(dst, tp[:])
                    else:
                        nc.scalar.copy(dst, tp[:])

                if si == 7:
                    for h in range(HPC):
                        pT = roam.tile([B, PT_COLS], BF16, tag="roam", bufs=4,
                                       name=f"pT{h}")
                        pTs.append(pT)
                for (h, kj, s1) in sched[si]:
                    _scores(h, kj, s1, chunked=True)
                    done.add((h, kj))
                qi_sched = {12: (0, 4), 13: (1, 5, 8), 14: (2, 6, 9),
                            15: (3, 7)}
                for qi in qi_sched.get(si, ()):
                    _chain_qi(qi)

            # si 0/1: hi-parts interleaved per weight batch, then lo-parts
            # paced by the wl stream, then si2/si3 at full speed
            EARLY_PE = []
            for b in range(4):
                EARLY_PE += [(1, 0, b, b + 1), (2, 0, b, b + 1)]
            for b in range(4):
                EARLY_PE += [(1, 1, b, b + 1), (2, 1, b, b + 1)]
            for b in range(4):
                EARLY_PE += [(3, 0, b, b + 1), (3, 1, b, b + 1)]
            EARLY_PE += [("fin", 0), ("fin", 1)]
            for ent in EARLY_PE:
                if ent[0] == "fin":
                    _finish_si(ent[1])
                else:
                    _qkv_part(ent[1], ent[0], ent[2], ent[3])
            for si in (2, 3):
                for term in (1, 2, 3):
                    _qkv_part(si, term, 0, 4)
                _finish_si(si)

            for si in range(4, NSB):
                t_ = work.tile([B, NCB, B], F8, tag="xh", bufs=3)
                nc.sync.dma_start(out=t_[:], in_=xh_h[si].bitcast(F8))
                early_xh[si] = t_
                t_ = work.tile([B, NCB, B], F8, tag="xl", bufs=3)
                nc.sync.dma_start(out=t_[:], in_=xl_h[si].bitcast(F8))
                early_xl[si] = t_
                t_ = work.tile([B, 512], F32, tag="cs", bufs=3)
                nc.sync.dma_start(out=t_[:], in_=cs_h[si])
                early_cs[si] = t_
                for term in (1, 2, 3):
                    _qkv_part(si, term, 0, 4)
                _finish_si(si)

            # wo reuses the wqkv slots; attnT is its own tensor so chains can
            # write it before the last qkv matmul retires
            woh_sb = bigp.tile([B, HPC, DIM], F8, tag="bigh")
            wol_sb = bigp.tile([B, HPC, DIM], F8, tag="bigl")
            for g in range(4):
                g0, g1 = g * 1024, (g + 1) * 1024
                for t in range(HPC // 2):
                    nc.sync.dma_start(out=woh_sb[:, 2 * t:2 * t + 2, g0:g1],
                                      in_=woh_h[t][:, :, g0:g1].bitcast(F8))
                for t in range(HPC // 2):
                    nc.sync.dma_start(out=wol_sb[:, 2 * t:2 * t + 2, g0:g1],
                                      in_=wol_h[t][:, :, g0:g1].bitcast(F8))

            # ===== Phase B2: PV + normalize, interleaved with wo per scol =====
            # seq0/seq1 chains (except 10/11) ran inside phase A; the
            # latency-bound leftover chains and seq2 scores/chains interleave
            # with wo(0)'s dense cp groups, covering the wo weight stream
            units = []
            for qi in (10, 11):
                for h in range(HPC):
                    units.append(("ch", qi, h))
            for kj in range(12, 16):
                for h in range(HPC):
                    units.append(("sc", kj, h))
                for h in range(HPC):
                    units.append(("ch", kj, h))
            cps = [(s, c) for s in range(3) for c in range(NCB)]
            nu, ncp = len(units), len(cps)
            ci = 0
            for i, u in enumerate(units):
                if u[0] == "ch":
                    _chain_head(u[1], u[2], tp_tag="B")
                else:
                    if (u[2], u[1]) not in done:
                        _scores(u[2], u[1], 16)
                take = (i + 1) * ncp // nu - i * ncp // nu
                for s, c in cps[ci:ci + take]:
                    _wo(s, [c])
                ci += take
            for s, c in cps[ci:]:
                _wo(s, [c])
            _wo(3)

    nc.finalize()
    return nc


def get_program():
    global _PROGRAM
    if _PROGRAM is None:
        _PROGRAM = _build_program()
    return _PROGRAM


def make_in_maps(x, cos, sin, wq, wk, wv, wo):
    import ml_dtypes
    f8 = ml_dtypes.float8_e4m3fn

    def split8(v):
        hi = v.astype(f8)
        lo = (v - hi.astype(np.float32)).astype(f8)
        return hi, lo

    x = np.asarray(x, np.float32)
    cos = np.asarray(cos, np.float32)
    sin = np.asarray(sin, np.float32)
    wq = np.asarray(wq, np.float32)
    wk = np.asarray(wk, np.float32)
    wv = np.asarray(wv, np.float32)
    wo = np.asarray(wo, np.float32)

    # xt[si, p, cb, s] = x[si*B + s, cb*B + p]
    xt = np.ascontiguousarray(
        x.reshape(NSB, B, NCB, B).transpose(0, 3, 2, 1)
    )
    xh, xl = split8(xt)
    # cos||sin tiled 4x along channels (per-head repeat), blocked by si,
    # divided by 512 to cancel the 512*sqrt(scale) on wq/wk (leaving sqrt(scale))
    cosr = np.tile(cos / 512.0, (1, HPC)).reshape(NSB, B, 2 * HEAD_DIM)
    sinr = np.tile(sin / 512.0, (1, HPC)).reshape(NSB, B, 2 * HEAD_DIM)
    csr = np.ascontiguousarray(np.concatenate([cosr, sinr], axis=2))
    # diagonal-block causal bias in scoresT layout: allow sq >= sk
    i = np.arange(B)
    dmask = np.where(i[None, :] >= i[:, None], 0.0, -30000.0).astype(np.float32)
    ident = np.eye(B, dtype=np.float32).astype(ml_dtypes.bfloat16)

    W768 = QW + 2 * KW
    in_maps = []
    for c in range(N_CORES):
        wq_c = wq[:, c * QW:(c + 1) * QW] * F_W
        wk_c = wk[:, c * KW:(c + 1) * KW] * F_W
        wv_c = wv[:, c * KW:(c + 1) * KW] * F_W
        wqkv_c = np.concatenate([wq_c, wk_c, wv_c], axis=1)
        # blocked [b, p, u, j] = wqkv[(8b+u)*B + p, j]
        wqkv_b = np.ascontiguousarray(
            wqkv_c.reshape(NCB // 8, 8, B, W768).transpose(0, 2, 1, 3))
        whb, wlb = split8(wqkv_b)
        wo_c = wo[c * QW:(c + 1) * QW, :] * E_WO
        wo_b = np.ascontiguousarray(
            wo_c.reshape(HPC // 2, 2, B, DIM).transpose(0, 2, 1, 3))
        woh, wol = split8(wo_b)
        in_maps.append({
            "xh": xh.view(np.uint8),
            "xl": xl.view(np.uint8),
            "whb": whb.view(np.uint8),
            "wlb": wlb.view(np.uint8),
            "woh": woh.view(np.uint8),
            "wol": wol.view(np.uint8),
            "csr": csr,
            "dmask": dmask,
            "ident": ident,
        })
    return in_maps


def combine_outputs(results):
    acc = np.zeros((NCB // 4, B, 4, 4, 512), np.float32)
    for r in results:
        acc += np.asarray(r["outp"]).astype(np.float32)
    acc *= 1.0 / E_WO
    # [cpg, p, j, scol, col] -> [cpg, j, p, scol, col] -> [DIM, S] -> [S, DIM]
    full = acc.transpose(0, 2, 1, 3, 4).reshape(DIM, S)
    return np.ascontiguousarray(full.T)


def kernel(x, cos, sin, mask, wq, wk, wv, wo):
    nc = get_program()
    in_maps = make_in_maps(x, cos, sin, wq, wk, wv, wo)
    res = run_bass_kernel_spmd(nc, in_maps, core_ids=list(range(N_CORES)))
    return combine_outputs(res.results)


# revision 34
# speedup vs baseline: 1.1940x; 1.0103x over previous
"""GQA sparse attention (packed seqs + sliding window + RoPE) on 8 Trainium2 cores.

Sharding: tensor-parallel over heads. Each of the 8 cores owns 4 Q-heads and
their single shared KV-head (GQA groups stay intact): wq columns
[h*512:(h+1)*512], wk/wv columns [h*128:(h+1)*128], wo rows [h*512:(h+1)*512].
Every core computes a full [S, DIM] partial of the output projection; the host
sums the 8 partials.

The mask never reaches the device: seqlens [1024, 512, 512] with causal +
sliding-window 1024 reduce to block-causal over 128-blocks within each
sequence (the window can never truncate since max causal span == 1024), plus
a causal bias on the diagonal 128x128 blocks.

The two dense projections (qkv, wo) run as fp8-e4m3 DoubleRow matmuls
(2 contraction rows per partition, 0.5 PE cycles per output column = 4x bf16
throughput) with a 3-term residual split for accuracy:
    A @ W  ~=  A_hi @ W_hi + A_lo @ W_hi + A_hi @ W_lo
where T_hi = fp8(T), T_lo = fp8(T - T_hi). Weight scales are arranged on the
host so every fp8 tensor sits in e4m3's sweet range: wq/wk/wv carry
512*sqrt(scale), rope's cos/sin carry 1/512 (leaving q,k scaled by
sqrt(scale) so scores come out exact), v's 512*sqrt(scale) cancels against
the softmax denominator by setting the appended ones-column to that same
constant, and wo carries 32 which the host divides back out.

Per-core dataflow (attention middle stays bf16 with fp32 PSUM):
  qkv:   psum[s,768] = sum_t sum_term x?[2t:2t+2].T @ w?[2t:2t+2]   (DR fp8)
  rope:  strided DVE ops on the psum, [s,d] layout, fp32 in / bf16 out
  qT/kT: PE transposes of the roped blocks
  scores(T): psum[sk, sq_span] = kT_blk.T @ qT[h]         (block-causal spans)
  p:     exp(scores + diag_bias) -> pT buffer, bf16
  pv:    psum[sq, 129] = sum_kj pT_blk.T @ [v_blk | ones*F]
  out:   attn = pv[:, :128] * recip(pv[:, 128]); transpose; split fp8 hi/lo
  wo:    psum[c',s] = sum_t sum_term wo?[2t:2t+2].T @ attnT?[2t:2t+2] (DR fp8)
"""

import os

os.environ.setdefault("JAX_PLATFORMS", "axon")

import numpy as np

import concourse.bass as bass
import concourse.mybir as mybir
import concourse.tile as tile
from concourse import bacc
from concourse.bass_utils import run_bass_kernel_spmd

# ---- problem constants (hardcoded per harness contract) ----
DIM = 4096
N_HEADS = 32
N_KV_HEADS = 8
HEAD_DIM = 128
SEQLENS = [1024, 512, 512]
S = 2048
N_CORES = 8
HPC = N_HEADS // N_CORES          # q heads per core = 4
QW = HPC * HEAD_DIM               # per-core q width = 512
KW = HEAD_DIM                     # per-core k/v width = 128
B = 128                           # block size
NSB = S // B                      # 16 seq blocks
NCB = DIM // B                    # 32 contraction blocks
SEQ_BLOCKS = []                   # [(start_blk, end_blk)] per packed sequence
_b = 0
for _l in SEQLENS:
    SEQ_BLOCKS.append((_b, _b + _l // B))
    _b += _l // B

# fp8 scale plumbing (see module docstring)
F_W = 512.0 * (HEAD_DIM ** -0.25)   # wq/wk/wv host scale; ones column value
E_WO = 32.0                         # wo host scale, divided out on host

# pT buffer layout: for each kj, columns [offs[kj] : offs[kj]+span(kj)) hold
# p.T for queries sq in [kj*B, seq_end)
_SPANS = {}
_OFFS = {}
_off = 0
for _s0, _s1 in SEQ_BLOCKS:
    for _kj in range(_s0, _s1):
        _SPANS[_kj] = (_s1 - _kj) * B
        _OFFS[_kj] = _off
        _off += _SPANS[_kj]
PT_COLS = _off                    # 7168

F32 = mybir.dt.float32
BF16 = mybir.dt.bfloat16
F8 = mybir.dt.float8e4
DR = mybir.MatmulPerfMode.DoubleRow

_PROGRAM = None


def _build_program():
    nc = bacc.Bacc(trn_type="TRN2")

    xh_h = nc.declare_dram_parameter("xh", [NSB, B, NCB, B], mybir.dt.uint8, isOutput=False)
    xl_h = nc.declare_dram_parameter("xl", [NSB, B, NCB, B], mybir.dt.uint8, isOutput=False)
    whb_h = nc.declare_dram_parameter("whb", [NCB // 8, B, 8, QW + 2 * KW], mybir.dt.uint8, isOutput=False)
    wlb_h = nc.declare_dram_parameter("wlb", [NCB // 8, B, 8, QW + 2 * KW], mybir.dt.uint8, isOutput=False)
    woh_h = nc.declare_dram_parameter("woh", [HPC // 2, B, 2, DIM], mybir.dt.uint8, isOutput=False)
    wol_h = nc.declare_dram_parameter("wol", [HPC // 2, B, 2, DIM], mybir.dt.uint8, isOutput=False)
    cs_h = nc.declare_dram_parameter("csr", [NSB, B, 4 * HEAD_DIM], F32, isOutput=False)
    dmask_h = nc.declare_dram_parameter("dmask", [B, B], F32, isOutput=False)
    ident_h = nc.declare_dram_parameter("ident", [B, B], BF16, isOutput=False)
    # out layout [cpg, p, j, scol, col]: a [B, 4, 512] SBUF tile DMAs to
    # outp[cpg, :, :, scol, :] with matching iteration order (host reassembles)
    out_h = nc.declare_dram_parameter("outp", [NCB // 4, B, 4, 4, 512], BF16, isOutput=True)

    W768 = QW + 2 * KW  # 768
    Exp = mybir.ActivationFunctionType.Exp

    with tile.TileContext(nc) as tc:
        with (
            tc.tile_pool(name="consts", bufs=1) as cpool,
            tc.tile_pool(name="big", bufs=1) as bigp,
            tc.tile_pool(name="persist", bufs=1) as pers,
            tc.tile_pool(name="roam", bufs=4) as roam,
            tc.tile_pool(name="work", bufs=3) as work,
            tc.tile_pool(name="psum", bufs=2, space="PSUM") as psum,
        ):
            # startup is DMA-bound: the stream order below is tuned so PE
            # always has runnable qkv work (see EARLY_DMA / EARLY_PE)
            wh_sb = bigp.tile([B, NCB, W768], F8, tag="bigh")
            wl_sb = bigp.tile([B, NCB, W768], F8, tag="bigl")
            early_xh = {}
            early_xl = {}
            early_cs = {}

            EARLY_DMA = [
                ("xhA", 0), ("wh", 0), ("xlA", 0), ("xhB", 0), ("wh", 1),
                ("xlB", 0), ("wh", 2), ("wh", 3), ("xh", 1), ("xl", 1),
                ("const",), ("cs", 0), ("cs", 1), ("wl", 0), ("wl", 1),
                ("wl", 2), ("wl", 3), ("xh", 2), ("xl", 2), ("cs", 2),
                ("xh", 3), ("xl", 3), ("cs", 3),
            ]
            ident_sb = cpool.tile([B, B], BF16)
            dmask_sb = cpool.tile([B, B], F32)
            for ent in EARLY_DMA:
                kind = ent[0]
                if kind in ("xhA", "xlA", "xhB", "xlB"):
                    si_, hl = ent[1], kind[1]
                    dram = xh_h if hl == "h" else xl_h
                    store = early_xh if hl == "h" else early_xl
                    if kind[2] == "A":
                        t_ = work.tile([B, NCB, B], F8, tag="x" + hl, bufs=3,
                                       name=f"x{hl}p{si_}")
                        store[si_] = t_
                        nc.sync.dma_start(out=t_[:, 0:16, :],
                                          in_=dram[si_][:, 0:16, :].bitcast(F8))
                    else:
                        t_ = store[si_]
                        nc.sync.dma_start(out=t_[:, 16:32, :],
                                          in_=dram[si_][:, 16:32, :].bitcast(F8))
                elif kind == "xh":
                    t_ = work.tile([B, NCB, B], F8, tag="xh", bufs=3,
                                   name=f"xhp{ent[1]}")
                    nc.sync.dma_start(out=t_[:], in_=xh_h[ent[1]].bitcast(F8))
                    early_xh[ent[1]] = t_
                elif kind == "xl":
                    t_ = work.tile([B, NCB, B], F8, tag="xl", bufs=3,
                                   name=f"xlp{ent[1]}")
                    nc.sync.dma_start(out=t_[:], in_=xl_h[ent[1]].bitcast(F8))
                    early_xl[ent[1]] = t_
                elif kind == "wh":
                    b = ent[1]
                    nc.sync.dma_start(out=wh_sb[:, 8 * b:8 * b + 8, :],
                                      in_=whb_h[b].bitcast(F8))
                elif kind == "wl":
                    b = ent[1]
                    nc.sync.dma_start(out=wl_sb[:, 8 * b:8 * b + 8, :],
                                      in_=wlb_h[b].bitcast(F8))
                elif kind == "cs":
                    t_ = work.tile([B, 512], F32, tag="cs", bufs=3,
                                   name=f"csp{ent[1]}")
                    nc.sync.dma_start(out=t_[:], in_=cs_h[ent[1]])
                    early_cs[ent[1]] = t_
                else:
                    nc.sync.dma_start(out=ident_sb[:], in_=ident_h[:])
                    nc.sync.dma_start(out=dmask_sb[:], in_=dmask_h[:])

            attnT_hi = pers.tile([B, HPC, S], F8)     # [d, head, seq]
            attnT_lo = pers.tile([B, HPC, S], F8)
            qT_sb = pers.tile([B, HPC * S], BF16)     # per head h: cols [h*S, (h+1)*S)
            kT_sb = pers.tile([B, S], BF16)
            vaug_sb = pers.tile([B, NSB * 129], BF16)  # per kj: [v_blk | ones*F_W]

            # scores + exp for one (head, kj) block-row
            pTs = []

            def _scores(h, kj, s1, chunked=False):
                pT = pTs[h]
                span = (s1 - kj) * B
                if chunked:
                    # phase-A-overlapped variant: 512-col chunks through tag B
                    # (PV's tag, idle during phase A) so the qkv psum pipeline
                    # in tag A is never paced by exp latency
                    for part in range(0, span, 512):
                        n = min(512, span - part)
                        ps_c = psum.tile([B, 512], F32, tag="B", bufs=2,
                                         name="ps_c")
                        nc.tensor.matmul(
                            ps_c[:, 0:n],
                            kT_sb[:, kj * B:(kj + 1) * B],
                            qT_sb[:, h * S + kj * B + part:
                                  h * S + kj * B + part + n],
                            start=True, stop=True,
                        )
                        if part == 0:
                            nc.vector.tensor_add(
                                ps_c[:, 0:B], ps_c[:, 0:B], dmask_sb[:]
                            )
                        nc.scalar.activation(
                            pT[:, _OFFS[kj] + part:_OFFS[kj] + part + n],
                            ps_c[:, 0:n], Exp
                        )
                    return
                ps_sc = psum.tile([B, 1024], F32, tag="A", bufs=2, name="ps_sc")
                for part in range(0, span, 512):
                    n = min(512, span - part)
                    nc.tensor.matmul(
                        ps_sc[:, part:part + n],
                        kT_sb[:, kj * B:(kj + 1) * B],
                        qT_sb[:, h * S + kj * B + part:
                              h * S + kj * B + part + n],
                        start=True, stop=True,
                    )
                # causal bias on the diagonal block
                nc.vector.tensor_add(ps_sc[:, 0:B], ps_sc[:, 0:B], dmask_sb[:])
                nc.scalar.activation(
                    pT[:, _OFFS[kj]:_OFFS[kj] + span], ps_sc[:, 0:span], Exp
                )

            # seq0/seq1 score groups overlap the back half of the qkv phase
            # (their qT/kT inputs are complete by then); seq2 runs after
            chains_done = set()
            sched = {si: [] for si in range(NSB)}
            for kj in range(0, 8):
                sched[8 + kj] = [(h, kj, 8) for h in range(HPC)]
            for kj, si in ((8, 12), (9, 13), (10, 14), (11, 15)):
                sched[si] += [(h, kj, 12) for h in range(HPC)]
            done = set()

            def _chain_head(qi, h, tp_tag="C"):
                    s0, s1 = next(b for b in SEQ_BLOCKS if b[0] <= qi < b[1])
                    if True:
                        pT = pTs[h]
                        ps_pv = psum.tile([B, 129], F32, tag="B", bufs=2)
                        for kj in range(s0, qi + 1):
                            lhsT = pT[:, _OFFS[kj] + (qi - kj) * B:
                                      _OFFS[kj] + (qi - kj + 1) * B]
                            nc.tensor.matmul(
                                ps_pv[:], lhsT,
                                vaug_sb[:, kj * 129:(kj + 1) * 129],
                                start=(kj == s0), stop=(kj == qi),
                            )
                        rc = work.tile([B, 1], F32, tag="rc", bufs=8)
                        nc.vector.reciprocal(rc[:], ps_pv[:, 128:129])
                        at = work.tile([B, B], BF16, tag="at", bufs=6)
                        nc.vector.tensor_scalar_mul(at[:], ps_pv[:, 0:B], rc[:])
                        tp = psum.tile([B, B], BF16, tag=tp_tag, bufs=2)
                        nc.tensor.transpose(tp[:], at[:], ident_sb[:])
                        hi_dst = attnT_hi[:, h, qi * B:(qi + 1) * B]
                        lo_dst = attnT_lo[:, h, qi * B:(qi + 1) * B]
                        nc.scalar.copy(hi_dst, tp[:])
                        nc.vector.tensor_sub(lo_dst, tp[:], hi_dst)

            ot_cur = {}

            def _chain_qi(qi, tp_tag="C"):
                chains_done.add(qi)
                for h in range(HPC):
                    _chain_head(qi, h, tp_tag)

            def _wo(scol, cps=None):
                for cp in (range(NCB) if cps is None else cps):
                    # alternate accumulators across tags C/A (the qkv psum in
                    # A is released by the stage-copy; B stays with chains)
                    if cp % 2 == 1:
                        psoF = psum.tile([B, 512], F32, tag="A", bufs=2,
                                         name="psoA")
                    else:
                        psoF = psum.tile([B, 512], F32, tag="C", bufs=2,
                                         name="psoC")
                    for sub in range(2):
                        pso = psoF[:, sub * 256:(sub + 1) * 256]
                        c0 = scol * 512 + sub * 256
                        nmm = 0
                        for t in range(HPC // 2):
                            for wa, aa in ((woh_sb, attnT_hi),
                                           (wol_sb, attnT_hi),
                                           (woh_sb, attnT_lo)):
                                nc.tensor.matmul(
                                    pso,
                                    wa[:, 2 * t:2 * t + 2, cp * B:(cp + 1) * B],
                                    aa[:, 2 * t:2 * t + 2, c0:c0 + 256],
                                    start=(nmm == 0), stop=(nmm == 5),
                                    perf_mode=DR,
                                )
                                nmm += 1
                    j = cp % 4
                    if scol == 3 and cp >= 28:
                        if j in (0, 2):
                            ot_cur[scol] = work.tile([B, 2, 512], BF16,
                                                     tag="ot1", bufs=2,
                                                     name="ott")
                        ot = ot_cur[scol]
                        if cp % 2 == 0:
                            nc.scalar.copy(ot[:, j % 2, :], psoF[:])
                        else:
                            nc.vector.tensor_copy(ot[:, j % 2, :], psoF[:])
                        if j in (1, 3):
                            nc.sync.dma_start(
                                out=out_h[cp // 4][:, j - 1:j + 1, scol, :],
                                in_=ot[:],
                            )
                        continue
                    if j == 0:
                        ot_cur[scol] = work.tile([B, 4, 512], BF16, tag="ot",
                                                 bufs=2, name="otb")
                    ot = ot_cur[scol]
                    if cp % 2 == 0:
                        nc.scalar.copy(ot[:, j, :], psoF[:])
                    else:
                        nc.vector.tensor_copy(ot[:, j, :], psoF[:])
                    if j == 3:
                        nc.sync.dma_start(
                            out=out_h[cp // 4][:, :, scol, :],
                            in_=ot[:],
                        )

            # =========== Phase A: qkv projection + rope + transposes ===========
            # Per si, the three residual terms split into a hi-part (terms
            # needing only the hi weights: xh@wh, xl@wh) and a lo-part
            # (xh@wl); the psum group opens at the first hi matmul and closes
            # at the last lo matmul. For si 0-3 the parts are emitted in a
            # hand-tuned order interleaved with the DMA stream; si 4-15 are
            # PE-bound and run parts back to back.
            ps_by_si = {}
            CHUNKS = ((512, 768), (0, 256), (256, 512))

            def _qkv_part(si, term, b0, b1):
                if si not in ps_by_si:
                    ps_by_si[si] = psum.tile([B, W768], F32, tag="A", bufs=2,
                                             name=f"ps{si % 2}")
                ps = ps_by_si[si]
                xa = early_xh[si] if term != 2 else early_xl[si]
                wa = wh_sb if term != 3 else wl_sb
                # chunks q0 [0:256] and q1 [256:512] share one 2KB psum
                # zero region: only q0's first matmul carries start=True (its
                # pending-zero covers q1's bytes); giving q1 its own start
                # while q0's group is still open would wipe q0's partials
                for c0, c1 in CHUNKS:
                    for t in range(4 * b0, 4 * b1):
                        nc.tensor.matmul(
                            ps[:, c0:c1],
                            xa[:, 2 * t:2 * t + 2, :],
                            wa[:, 2 * t:2 * t + 2, c0:c1],
                            start=(term == 1 and t == 0 and c0 != 256),
                            stop=(term == 3 and t == NCB // 2 - 1),
                            perf_mode=DR,
                            skip_group_check=True,
                        )

            def _finish_si(si):
                psum_ps = ps_by_si.pop(si)
                cs_t = early_cs.pop(si)
                cs = cs_t[:, 0:256]
                sn = cs_t[:, 256:512]

                # stage the qkv psum to SBUF in one fast Act copy: releases
                # the psum buffer for si+2 immediately, and the strided rope
                # reads below hit SBUF (58-cycle access) instead of PSUM (120)
                ps = work.tile([B, W768], F32, tag="qkvs", bufs=2)
                nc.scalar.copy(ps[:], psum_ps[:])

                # rope on k first (kT feeds the phase-A score groups)
                k_t = work.tile([B, KW], BF16, tag="k", bufs=3)
                ke, ko = ps[:, 512:640:2], ps[:, 513:640:2]
                c64, s64 = cs_t[:, 0:64], cs_t[:, 256:320]
                u1 = work.tile([B, 64], F32, tag="u1", bufs=2)
                u2 = work.tile([B, 64], F32, tag="u2", bufs=2)
                u3 = work.tile([B, 64], F32, tag="u1", bufs=2, name="u3")
                u4 = work.tile([B, 64], F32, tag="u2", bufs=2, name="u4")
                nc.vector.tensor_mul(u1[:], ke, c64)
                nc.vector.tensor_mul(u2[:], ko, s64)
                nc.vector.tensor_sub(k_t[:, 0:KW:2], u1[:], u2[:])
                nc.vector.tensor_mul(u3[:], ke, s64)
                nc.vector.tensor_mul(u4[:], ko, c64)
                nc.vector.tensor_add(k_t[:, 1:KW:2], u3[:], u4[:])

                # v block + ones column (ones = F_W cancels v's host scale)
                nc.scalar.copy(vaug_sb[:, si * 129:si * 129 + 128], ps[:, 640:768])
                nc.vector.memset(vaug_sb[:, si * 129 + 128:si * 129 + 129], F_W)

                # rope on q: [s, d] layout, channels interleaved (even, odd)
                q_t = work.tile([B, QW], BF16, tag="q", bufs=3)
                qe, qo = ps[:, 0:QW:2], ps[:, 1:QW:2]
                t1 = work.tile([B, 256], F32, tag="t1", bufs=2)
                t2 = work.tile([B, 256], F32, tag="t2", bufs=2)
                t3 = work.tile([B, 256], F32, tag="t1", bufs=2, name="t3")
                t4 = work.tile([B, 256], F32, tag="t2", bufs=2, name="t4")
                nc.vector.tensor_mul(t1[:], qe, cs)
                nc.vector.tensor_mul(t2[:], qo, sn)
                nc.vector.tensor_sub(q_t[:, 0:QW:2], t1[:], t2[:])
                nc.vector.tensor_mul(t3[:], qe, sn)
                nc.vector.tensor_mul(t4[:], qo, cs)
                nc.vector.tensor_add(q_t[:, 1:QW:2], t3[:], t4[:])

                # transposes: k first (feeds scores), then q (4 blocks)
                ktp = psum.tile([B, B], BF16, tag="C", bufs=2)
                nc.tensor.transpose(ktp[:], k_t[:], ident_sb[:])
                nc.vector.tensor_copy(kT_sb[:, si * B:(si + 1) * B], ktp[:])
                for h in range(HPC):
                    tp = psum.tile([B, B], BF16, tag="C", bufs=2)
                    nc.tensor.transpose(tp[:], q_t[:, h * B:(h + 1) * B], ident_sb[:])
                    dst = qT_sb[:, h * S + si * B:h * S + (si + 1) * B]
                    if h % 2 == 0:
                        nc.vector.tensor_copy(dst, tp[:])
                    else:
                        nc.scalar.copy(dst, tp[:])

                if si == 7:
                    for h in range(HPC):
                        pT = roam.tile([B, PT_COLS], BF16, tag="roam", bufs=4,
                                       name=f"pT{h}")
                        pTs.append(pT)
                for (h, kj, s1) in sched[si]:
                    _scores(h, kj, s1, chunked=True)
                    done.add((h, kj))
                qi_sched = {12: (0, 4), 13: (1, 5, 8), 14: (2, 6, 9),
                            15: (3, 7)}
                for qi in qi_sched.get(si, ()):
                    _chain_qi(qi)

            # si 0/1: hi-parts interleaved per weight batch, then lo-parts
            # paced by the wl stream, then si2/si3 at full speed
            EARLY_PE = []
            for b in range(4):
                EARLY_PE += [(1, 0, b, b + 1), (2, 0, b, b + 1)]
            for b in range(4):
                EARLY_PE += [(1, 1, b, b + 1), (2, 1, b, b + 1)]
            for b in range(4):
                EARLY_PE += [(3, 0, b, b + 1), (3, 1, b, b + 1)]
            EARLY_PE += [("fin", 0), ("fin", 1)]
            for ent in EARLY_PE:
                if ent[0] == "fin":
                    _finish_si(ent[1])
                else:
                    _qkv_part(ent[1], ent[0], ent[2], ent[3])
            for si in (2, 3):
                for term in (1, 2, 3):
                    _qkv_part(si, term, 0, 4)
                _finish_si(si)

            for si in range(4, NSB):
                t_ = work.tile([B, NCB, B], F8, tag="xh", bufs=3)
                nc.sync.dma_start(out=t_[:], in_=xh_h[si].bitcast(F8))
                early_xh[si] = t_
                t_ = work.tile([B, NCB, B], F8, tag="xl", bufs=3)
                nc.sync.dma_start(out=t_[:], in_=xl_h[si].bitcast(F8))
                early_xl[si] = t_
                t_ = work.tile([B, 512], F32, tag="cs", bufs=3)
                nc.sync.dma_start(out=t_[:], in_=cs_h[si])
                early_cs[si] = t_
                for term in (1, 2, 3):
                    _qkv_part(si, term, 0, 4)
                _finish_si(si)

            # wo reuses the wqkv slots; attnT is its own tensor so chains can
            # write it before the last qkv matmul retires
            woh_sb = bigp.tile([B, HPC, DIM], F8, tag="bigh")
            wol_sb = bigp.tile([B, HPC, DIM], F8, tag="bigl")
            for g in range(4):
                g0, g1 = g * 1024, (g + 1) * 1024
                for t in range(HPC // 2):
                    nc.sync.dma_start(out=woh_sb[:, 2 * t:2 * t + 2, g0:g1],
                                      in_=woh_h[t][:, :, g0:g1].bitcast(F8))
                for t in range(HPC // 2):
                    nc.sync.dma_start(out=wol_sb[:, 2 * t:2 * t + 2, g0:g1],
                                      in_=wol_h[t][:, :, g0:g1].bitcast(F8))

            # ===== Phase B2: PV + normalize, interleaved with wo per scol =====
            # seq0/seq1 chains (except 10/11) ran inside phase A; the
            # latency-bound leftover chains and seq2 scores/chains interleave
            # with wo(0)'s dense cp groups, covering the wo weight stream
            units = []
            for kj in range(12, 16):
                for h in range(HPC):
                    units.append(("sc", kj, h))
            for qi in (10, 11, 12, 13, 14, 15):
                for h in range(HPC):
                    units.append(("ch", qi, h))
            cps = [(s, c) for s in range(3) for c in range(NCB)]
            nu, ncp = len(units), len(cps)
            ci = 0
            RAMP = 6   # no cps among the first units: their wo weights are
                       # still streaming and a blocked cp stalls the in-order
                       # PE queue behind it
            for i, u in enumerate(units):
                if u[0] == "ch":
                    _chain_head(u[1], u[2], tp_tag="B")
                else:
                    if (u[2], u[1]) not in done:
                        _scores(u[2], u[1], 16)
                if i < RAMP:
                    continue
                take = ((i + 1 - RAMP) * ncp) // (nu - RAMP) - ((i - RAMP) * ncp) // (nu - RAMP)
                for s, c in cps[ci:ci + take]:
                    _wo(s, [c])
                ci += take
            for s, c in cps[ci:]:
                _wo(s, [c])
            _wo(3)

    nc.finalize()
    return nc


def get_program():
    global _PROGRAM
    if _PROGRAM is None:
        _PROGRAM = _build_program()
    return _PROGRAM


def make_in_maps(x, cos, sin, wq, wk, wv, wo):
    import ml_dtypes
    f8 = ml_dtypes.float8_e4m3fn

    def split8(v):
        hi = v.astype(f8)
        lo = (v - hi.astype(np.float32)).astype(f8)
        return hi, lo

    x = np.asarray(x, np.float32)
    cos = np.asarray(cos, np.float32)
    sin = np.asarray(sin, np.float32)
    wq = np.asarray(wq, np.float32)
    wk = np.asarray(wk, np.float32)
    wv = np.asarray(wv, np.float32)
    wo = np.asarray(wo, np.float32)

    # xt[si, p, cb, s] = x[si*B + s, cb*B + p]
    xt = np.ascontiguousarray(
        x.reshape(NSB, B, NCB, B).transpose(0, 3, 2, 1)
    )
    xh, xl = split8(xt)
    # cos||sin tiled 4x along channels (per-head repeat), blocked by si,
    # divided by 512 to cancel the 512*sqrt(scale) on wq/wk (leaving sqrt(scale))
    cosr = np.tile(cos / 512.0, (1, HPC)).reshape(NSB, B, 2 * HEAD_DIM)
    sinr = np.tile(sin / 512.0, (1, HPC)).reshape(NSB, B, 2 * HEAD_DIM)
    csr = np.ascontiguousarray(np.concatenate([cosr, sinr], axis=2))
    # diagonal-block causal bias in scoresT layout: allow sq >= sk
    i = np.arange(B)
    dmask = np.where(i[None, :] >= i[:, None], 0.0, -30000.0).astype(np.float32)
    ident = np.eye(B, dtype=np.float32).astype(ml_dtypes.bfloat16)

    W768 = QW + 2 * KW
    in_maps = []
    for c in range(N_CORES):
        wq_c = wq[:, c * QW:(c + 1) * QW] * F_W
        wk_c = wk[:, c * KW:(c + 1) * KW] * F_W
        wv_c = wv[:, c * KW:(c + 1) * KW] * F_W
        wqkv_c = np.concatenate([wq_c, wk_c, wv_c], axis=1)
        # blocked [b, p, u, j] = wqkv[(8b+u)*B + p, j]
        wqkv_b = np.ascontiguousarray(
            wqkv_c.reshape(NCB // 8, 8, B, W768).transpose(0, 2, 1, 3))
        whb, wlb = split8(wqkv_b)
        wo_c = wo[c * QW:(c + 1) * QW, :] * E_WO
        wo_b = np.ascontiguousarray(
            wo_c.reshape(HPC // 2, 2, B, DIM).transpose(0, 2, 1, 3))
        woh, wol = split8(wo_b)
        in_maps.append({
            "xh": xh.view(np.uint8),
            "xl": xl.view(np.uint8),
            "whb": whb.view(np.uint8),
            "wlb": wlb.view(np.uint8),
            "woh": woh.view(np.uint8),
            "wol": wol.view(np.uint8),
            "csr": csr,
            "dmask": dmask,
            "ident": ident,
        })
    return in_maps


def combine_outputs(results):
    acc = np.zeros((NCB // 4, B, 4, 4, 512), np.float32)
    for r in results:
        acc += np.asarray(r["outp"]).astype(np.float32)
    acc *= 1.0 / E_WO
    # [cpg, p, j, scol, col] -> [cpg, j, p, scol, col] -> [DIM, S] -> [S, DIM]
    full = acc.transpose(0, 2, 1, 3, 4).reshape(DIM, S)
    return np.ascontiguousarray(full.T)


def kernel(x, cos, sin, mask, wq, wk, wv, wo):
    nc = get_program()
    in_maps = make_in_maps(x, cos, sin, wq, wk, wv, wo)
    res = run_bass_kernel_spmd(nc, in_maps, core_ids=list(range(N_CORES)))
    return combine_outputs(res.results)


# revision 44
# speedup vs baseline: 1.2007x; 1.0056x over previous
"""GQA sparse attention (packed seqs + sliding window + RoPE) on 8 Trainium2 cores.

Sharding: tensor-parallel over heads. Each of the 8 cores owns 4 Q-heads and
their single shared KV-head (GQA groups stay intact): wq columns
[h*512:(h+1)*512], wk/wv columns [h*128:(h+1)*128], wo rows [h*512:(h+1)*512].
Every core computes a full [S, DIM] partial of the output projection; the host
sums the 8 partials.

The mask never reaches the device: seqlens [1024, 512, 512] with causal +
sliding-window 1024 reduce to block-causal over 128-blocks within each
sequence (the window can never truncate since max causal span == 1024), plus
a causal bias on the diagonal 128x128 blocks.

The two dense projections (qkv, wo) run as fp8-e4m3 DoubleRow matmuls
(2 contraction rows per partition, 0.5 PE cycles per output column = 4x bf16
throughput) with a 3-term residual split for accuracy:
    A @ W  ~=  A_hi @ W_hi + A_lo @ W_hi + A_hi @ W_lo
where T_hi = fp8(T), T_lo = fp8(T - T_hi). Weight scales are arranged on the
host so every fp8 tensor sits in e4m3's sweet range: wq/wk/wv carry
512*sqrt(scale), rope's cos/sin carry 1/512 (leaving q,k scaled by
sqrt(scale) so scores come out exact), v's 512*sqrt(scale) cancels against
the softmax denominator by setting the appended ones-column to that same
constant, and wo carries 32 which the host divides back out.

Per-core dataflow (attention middle stays bf16 with fp32 PSUM):
  qkv:   psum[s,768] = sum_t sum_term x?[2t:2t+2].T @ w?[2t:2t+2]   (DR fp8)
  rope:  strided DVE ops on the psum, [s,d] layout, fp32 in / bf16 out
  qT/kT: PE transposes of the roped blocks
  scores(T): psum[sk, sq_span] = kT_blk.T @ qT[h]         (block-causal spans)
  p:     exp(scores + diag_bias) -> pT buffer, bf16
  pv:    psum[sq, 129] = sum_kj pT_blk.T @ [v_blk | ones*F]
  out:   attn = pv[:, :128] * recip(pv[:, 128]); transpose; split fp8 hi/lo
  wo:    psum[c',s] = sum_t sum_term wo?[2t:2t+2].T @ attnT?[2t:2t+2] (DR fp8)
"""

import os

os.environ.setdefault("JAX_PLATFORMS", "axon")

import numpy as np

import concourse.bass as bass
import concourse.mybir as mybir
import concourse.tile as tile
from concourse import bacc
from concourse.bass_utils import run_bass_kernel_spmd

# ---- problem constants (hardcoded per harness contract) ----
DIM = 4096
N_HEADS = 32
N_KV_HEADS = 8
HEAD_DIM = 128
SEQLENS = [1024, 512, 512]
S = 2048
N_CORES = 8
HPC = N_HEADS // N_CORES          # q heads per core = 4
QW = HPC * HEAD_DIM               # per-core q width = 512
KW = HEAD_DIM                     # per-core k/v width = 128
B = 128                           # block size
NSB = S // B                      # 16 seq blocks
NCB = DIM // B                    # 32 contraction blocks
SEQ_BLOCKS = []                   # [(start_blk, end_blk)] per packed sequence
_b = 0
for _l in SEQLENS:
    SEQ_BLOCKS.append((_b, _b + _l // B))
    _b += _l // B

# fp8 scale plumbing (see module docstring)
F_W = 512.0 * (HEAD_DIM ** -0.25)   # wq/wk/wv host scale; ones column value
E_WO = 32.0                         # wo host scale, divided out on host

# pT buffer layout: for each kj, columns [offs[kj] : offs[kj]+span(kj)) hold
# p.T for queries sq in [kj*B, seq_end)
_SPANS = {}
_OFFS = {}
_off = 0
for _s0, _s1 in SEQ_BLOCKS:
    for _kj in range(_s0, _s1):
        _SPANS[_kj] = (_s1 - _kj) * B
        _OFFS[_kj] = _off
        _off += _SPANS[_kj]
PT_COLS = _off                    # 7168

F32 = mybir.dt.float32
BF16 = mybir.dt.bfloat16
F8 = mybir.dt.float8e4
DR = mybir.MatmulPerfMode.DoubleRow

_PROGRAM = None


def _build_program():
    nc = bacc.Bacc(trn_type="TRN2")

    xh_h = nc.declare_dram_parameter("xh", [NSB, B, NCB, B], mybir.dt.uint8, isOutput=False)
    xl_h = nc.declare_dram_parameter("xl", [NSB, B, NCB, B], mybir.dt.uint8, isOutput=False)
    whb_h = nc.declare_dram_parameter("whb", [NCB // 8, B, 8, QW + 2 * KW], mybir.dt.uint8, isOutput=False)
    wlb_h = nc.declare_dram_parameter("wlb", [NCB // 8, B, 8, QW + 2 * KW], mybir.dt.uint8, isOutput=False)
    woh_h = nc.declare_dram_parameter("woh", [HPC // 2, B, 2, DIM], mybir.dt.uint8, isOutput=False)
    wol_h = nc.declare_dram_parameter("wol", [HPC // 2, B, 2, DIM], mybir.dt.uint8, isOutput=False)
    cs_h = nc.declare_dram_parameter("csr", [NSB, B, 4 * HEAD_DIM], F32, isOutput=False)
    dmask_h = nc.declare_dram_parameter("dmask", [B, B], F32, isOutput=False)
    ident_h = nc.declare_dram_parameter("ident", [B, B], BF16, isOutput=False)
    # out layout [cpg, p, j, scol, col]: a [B, 4, 512] SBUF tile DMAs to
    # outp[cpg, :, :, scol, :] with matching iteration order (host reassembles)
    out_h = nc.declare_dram_parameter("outp", [NCB // 4, B, 4, 4, 512], BF16, isOutput=True)

    W768 = QW + 2 * KW  # 768
    Exp = mybir.ActivationFunctionType.Exp

    with tile.TileContext(nc) as tc:
        with (
            tc.tile_pool(name="consts", bufs=1) as cpool,
            tc.tile_pool(name="big", bufs=1) as bigp,
            tc.tile_pool(name="persist", bufs=1) as pers,
            tc.tile_pool(name="roam", bufs=4) as roam,
            tc.tile_pool(name="work", bufs=3) as work,
            tc.tile_pool(name="psum", bufs=2, space="PSUM") as psum,
        ):
            # startup is DMA-bound: the stream order below is tuned so PE
            # always has runnable qkv work (see EARLY_DMA / EARLY_PE)
            wh_sb = bigp.tile([B, NCB, W768], F8, tag="bigh")
            wl_sb = bigp.tile([B, NCB, W768], F8, tag="bigl")
            early_xh = {}
            early_xl = {}
            early_cs = {}

            EARLY_DMA = [
                ("xhA", 0), ("wh", 0), ("xlA", 0), ("xhB", 0), ("wh", 1),
                ("xlB", 0), ("wh", 2), ("wh", 3), ("xh", 1), ("xl", 1),
                ("const",), ("cs", 0), ("cs", 1), ("wl", 0), ("wl", 1),
                ("wl", 2), ("wl", 3), ("xh", 2), ("xl", 2), ("cs", 2),
                ("xh", 3), ("xl", 3), ("cs", 3),
            ]
            ident_sb = cpool.tile([B, B], BF16)
            dmask_sb = cpool.tile([B, B], F32)
            for ent in EARLY_DMA:
                kind = ent[0]
                if kind in ("xhA", "xlA", "xhB", "xlB"):
                    si_, hl = ent[1], kind[1]
                    dram = xh_h if hl == "h" else xl_h
                    store = early_xh if hl == "h" else early_xl
                    if kind[2] == "A":
                        t_ = work.tile([B, NCB, B], F8, tag="x" + hl, bufs=3,
                                       name=f"x{hl}p{si_}")
                        store[si_] = t_
                        nc.sync.dma_start(out=t_[:, 0:16, :],
                                          in_=dram[si_][:, 0:16, :].bitcast(F8))
                    else:
                        t_ = store[si_]
                        nc.sync.dma_start(out=t_[:, 16:32, :],
                                          in_=dram[si_][:, 16:32, :].bitcast(F8))
                elif kind == "xh":
                    t_ = work.tile([B, NCB, B], F8, tag="xh", bufs=3,
                                   name=f"xhp{ent[1]}")
                    nc.sync.dma_start(out=t_[:], in_=xh_h[ent[1]].bitcast(F8))
                    early_xh[ent[1]] = t_
                elif kind == "xl":
                    t_ = work.tile([B, NCB, B], F8, tag="xl", bufs=3,
                                   name=f"xlp{ent[1]}")
                    nc.sync.dma_start(out=t_[:], in_=xl_h[ent[1]].bitcast(F8))
                    early_xl[ent[1]] = t_
                elif kind == "wh":
                    b = ent[1]
                    nc.sync.dma_start(out=wh_sb[:, 8 * b:8 * b + 8, :],
                                      in_=whb_h[b].bitcast(F8))
                elif kind == "wl":
                    b = ent[1]
                    nc.sync.dma_start(out=wl_sb[:, 8 * b:8 * b + 8, :],
                                      in_=wlb_h[b].bitcast(F8))
                elif kind == "cs":
                    t_ = work.tile([B, 512], F32, tag="cs", bufs=3,
                                   name=f"csp{ent[1]}")
                    nc.sync.dma_start(out=t_[:], in_=cs_h[ent[1]])
                    early_cs[ent[1]] = t_
                else:
                    nc.sync.dma_start(out=ident_sb[:], in_=ident_h[:])
                    nc.sync.dma_start(out=dmask_sb[:], in_=dmask_h[:])

            attnT_hi = pers.tile([B, HPC, S], F8)     # [d, head, seq]
            attnT_lo = pers.tile([B, HPC, S], F8)
            qT_sb = pers.tile([B, HPC * S], BF16)     # per head h: cols [h*S, (h+1)*S)
            kT_sb = pers.tile([B, S], BF16)
            vaug_sb = pers.tile([B, NSB * 129], BF16)  # per kj: [v_blk | ones*F_W]

            # scores + exp for one (head, kj) block-row
            pTs = []

            def _scores(h, kj, s1, chunked=False):
                pT = pTs[h]
                span = (s1 - kj) * B
                if chunked:
                    # phase-A-overlapped variant: 512-col chunks through tag B
                    # (PV's tag, idle during phase A) so the qkv psum pipeline
                    # in tag A is never paced by exp latency
                    for part in range(0, span, 512):
                        n = min(512, span - part)
                        ps_c = psum.tile([B, 512], F32, tag="B", bufs=2,
                                         name="ps_c")
                        nc.tensor.matmul(
                            ps_c[:, 0:n],
                            kT_sb[:, kj * B:(kj + 1) * B],
                            qT_sb[:, h * S + kj * B + part:
                                  h * S + kj * B + part + n],
                            start=True, stop=True,
                        )
                        if part == 0:
                            nc.vector.tensor_add(
                                ps_c[:, 0:B], ps_c[:, 0:B], dmask_sb[:]
                            )
                        nc.scalar.activation(
                            pT[:, _OFFS[kj] + part:_OFFS[kj] + part + n],
                            ps_c[:, 0:n], Exp
                        )
                    return
                ps_sc = psum.tile([B, 1024], F32, tag="A", bufs=2, name="ps_sc")
                for part in range(0, span, 512):
                    n = min(512, span - part)
                    nc.tensor.matmul(
                        ps_sc[:, part:part + n],
                        kT_sb[:, kj * B:(kj + 1) * B],
                        qT_sb[:, h * S + kj * B + part:
                              h * S + kj * B + part + n],
                        start=True, stop=True,
                    )
                # causal bias on the diagonal block
                nc.vector.tensor_add(ps_sc[:, 0:B], ps_sc[:, 0:B], dmask_sb[:])
                nc.scalar.activation(
                    pT[:, _OFFS[kj]:_OFFS[kj] + span], ps_sc[:, 0:span], Exp
                )

            # seq0/seq1 score groups overlap the back half of the qkv phase
            # (their qT/kT inputs are complete by then); seq2 runs after
            chains_done = set()
            sched = {si: [] for si in range(NSB)}
            for kj in range(0, 8):
                sched[8 + kj] = [(h, kj, 8) for h in range(HPC)]
            for kj, si in ((8, 12), (9, 13), (10, 14), (11, 15)):
                sched[si] += [(h, kj, 12) for h in range(HPC)]
            done = set()

            def _chain_head(qi, h, tp_tag="C"):
                    s0, s1 = next(b for b in SEQ_BLOCKS if b[0] <= qi < b[1])
                    if True:
                        pT = pTs[h]
                        ps_pv = psum.tile([B, 129], F32, tag="B", bufs=2)
                        for kj in range(s0, qi + 1):
                            lhsT = pT[:, _OFFS[kj] + (qi - kj) * B:
                                      _OFFS[kj] + (qi - kj + 1) * B]
                            nc.tensor.matmul(
                                ps_pv[:], lhsT,
                                vaug_sb[:, kj * 129:(kj + 1) * 129],
                                start=(kj == s0), stop=(kj == qi),
                            )
                        rc = work.tile([B, 1], F32, tag="rc", bufs=8)
                        nc.vector.reciprocal(rc[:], ps_pv[:, 128:129])
                        at = work.tile([B, B], BF16, tag="at", bufs=6)
                        nc.vector.tensor_scalar_mul(at[:], ps_pv[:, 0:B], rc[:])
                        tp = psum.tile([B, B], BF16, tag=tp_tag, bufs=2)
                        nc.tensor.transpose(tp[:], at[:], ident_sb[:])
                        hi_dst = attnT_hi[:, h, qi * B:(qi + 1) * B]
                        lo_dst = attnT_lo[:, h, qi * B:(qi + 1) * B]
                        nc.scalar.copy(hi_dst, tp[:])
                        nc.vector.tensor_sub(lo_dst, tp[:], hi_dst)

            ot_cur = {}

            def _chain_qi(qi, tp_tag="C"):
                chains_done.add(qi)
                for h in range(HPC):
                    _chain_head(qi, h, tp_tag)

            def _wo(scol, cps=None):
                for cp in (range(NCB) if cps is None else cps):
                    # alternate accumulators across tags C/A (the qkv psum in
                    # A is released by the stage-copy; B stays with chains)
                    if cp % 2 == 1:
                        psoF = psum.tile([B, 512], F32, tag="A", bufs=2,
                                         name="psoA")
                    else:
                        psoF = psum.tile([B, 512], F32, tag="C", bufs=2,
                                         name="psoC")
                    for sub in range(2):
                        pso = psoF[:, sub * 256:(sub + 1) * 256]
                        c0 = scol * 512 + sub * 256
                        nmm = 0
                        for t in range(HPC // 2):
                            for wa, aa in ((woh_sb, attnT_hi),
                                           (wol_sb, attnT_hi),
                                           (woh_sb, attnT_lo)):
                                nc.tensor.matmul(
                                    pso,
                                    wa[:, 2 * t:2 * t + 2, cp * B:(cp + 1) * B],
                                    aa[:, 2 * t:2 * t + 2, c0:c0 + 256],
                                    start=(nmm == 0), stop=(nmm == 5),
                                    perf_mode=DR,
                                )
                                nmm += 1
                    j = cp % 4
                    if scol == 3 and cp >= 28:
                        if j in (0, 2):
                            ot_cur[scol] = work.tile([B, 2, 512], BF16,
                                                     tag="ot1", bufs=2,
                                                     name="ott")
                        ot = ot_cur[scol]
                        if cp % 2 == 0:
                            nc.scalar.copy(ot[:, j % 2, :], psoF[:])
                        else:
                            nc.vector.tensor_copy(ot[:, j % 2, :], psoF[:])
                        if j in (1, 3):
                            nc.sync.dma_start(
                                out=out_h[cp // 4][:, j - 1:j + 1, scol, :],
                                in_=ot[:],
                            )
                        continue
                    if j == 0:
                        ot_cur[scol] = work.tile([B, 4, 512], BF16, tag="ot",
                                                 bufs=2, name="otb")
                    ot = ot_cur[scol]
                    if cp % 2 == 0:
                        nc.scalar.copy(ot[:, j, :], psoF[:])
                    else:
                        nc.vector.tensor_copy(ot[:, j, :], psoF[:])
                    if j == 3:
                        nc.sync.dma_start(
                            out=out_h[cp // 4][:, :, scol, :],
                            in_=ot[:],
                        )

            # =========== Phase A: qkv projection + rope + transposes ===========
            # Per si, the three residual terms split into a hi-part (terms
            # needing only the hi weights: xh@wh, xl@wh) and a lo-part
            # (xh@wl); the psum group opens at the first hi matmul and closes
            # at the last lo matmul. For si 0-3 the parts are emitted in a
            # hand-tuned order interleaved with the DMA stream; si 4-15 are
            # PE-bound and run parts back to back.
            ps_by_si = {}
            CHUNKS = ((512, 768), (0, 256), (256, 512))

            def _qkv_part(si, term, b0, b1):
                if si not in ps_by_si:
                    ps_by_si[si] = psum.tile([B, W768], F32, tag="A", bufs=2,
                                             name=f"ps{si % 2}")
                ps = ps_by_si[si]
                xa = early_xh[si] if term != 2 else early_xl[si]
                wa = wh_sb if term != 3 else wl_sb
                # chunks q0 [0:256] and q1 [256:512] share one 2KB psum
                # zero region: only q0's first matmul carries start=True (its
                # pending-zero covers q1's bytes); giving q1 its own start
                # while q0's group is still open would wipe q0's partials
                for c0, c1 in CHUNKS:
                    for t in range(4 * b0, 4 * b1):
                        nc.tensor.matmul(
                            ps[:, c0:c1],
                            xa[:, 2 * t:2 * t + 2, :],
                            wa[:, 2 * t:2 * t + 2, c0:c1],
                            start=(term == 1 and t == 0 and c0 != 256),
                            stop=(term == 3 and t == NCB // 2 - 1),
                            perf_mode=DR,
                            skip_group_check=True,
                        )

            def _finish_si(si):
                psum_ps = ps_by_si.pop(si)
                cs_t = early_cs.pop(si)
                cs = cs_t[:, 0:256]
                sn = cs_t[:, 256:512]

                # stage the qkv psum to SBUF in one fast Act copy: releases
                # the psum buffer for si+2 immediately, and the strided rope
                # reads below hit SBUF (58-cycle access) instead of PSUM (120)
                ps = work.tile([B, W768], F32, tag="qkvs", bufs=2)
                nc.scalar.copy(ps[:], psum_ps[:])

                # rope on k first (kT feeds the phase-A score groups)
                k_t = work.tile([B, KW], BF16, tag="k", bufs=3)
                ke, ko = ps[:, 512:640:2], ps[:, 513:640:2]
                c64, s64 = cs_t[:, 0:64], cs_t[:, 256:320]
                u1 = work.tile([B, 64], F32, tag="u1", bufs=2)
                u2 = work.tile([B, 64], F32, tag="u2", bufs=2)
                u3 = work.tile([B, 64], F32, tag="u1", bufs=2, name="u3")
                u4 = work.tile([B, 64], F32, tag="u2", bufs=2, name="u4")
                nc.vector.tensor_mul(u1[:], ke, c64)
                nc.vector.tensor_mul(u2[:], ko, s64)
                nc.vector.tensor_sub(k_t[:, 0:KW:2], u1[:], u2[:])
                nc.vector.tensor_mul(u3[:], ke, s64)
                nc.vector.tensor_mul(u4[:], ko, c64)
                nc.vector.tensor_add(k_t[:, 1:KW:2], u3[:], u4[:])

                # v block + ones column (ones = F_W cancels v's host scale)
                nc.scalar.copy(vaug_sb[:, si * 129:si * 129 + 128], ps[:, 640:768])
                nc.vector.memset(vaug_sb[:, si * 129 + 128:si * 129 + 129], F_W)

                # rope on q: [s, d] layout, channels interleaved (even, odd)
                q_t = work.tile([B, QW], BF16, tag="q", bufs=3)
                qe, qo = ps[:, 0:QW:2], ps[:, 1:QW:2]
                t1 = work.tile([B, 256], F32, tag="t1", bufs=2)
                t2 = work.tile([B, 256], F32, tag="t2", bufs=2)
                t3 = work.tile([B, 256], F32, tag="t1", bufs=2, name="t3")
                t4 = work.tile([B, 256], F32, tag="t2", bufs=2, name="t4")
                nc.vector.tensor_mul(t1[:], qe, cs)
                nc.vector.tensor_mul(t2[:], qo, sn)
                nc.vector.tensor_sub(q_t[:, 0:QW:2], t1[:], t2[:])
                nc.vector.tensor_mul(t3[:], qe, sn)
                nc.vector.tensor_mul(t4[:], qo, cs)
                nc.vector.tensor_add(q_t[:, 1:QW:2], t3[:], t4[:])

                # transposes: k first (feeds scores), then q (4 blocks)
                ktp = psum.tile([B, B], BF16, tag="C", bufs=2)
                nc.tensor.transpose(ktp[:], k_t[:], ident_sb[:])
                nc.vector.tensor_copy(kT_sb[:, si * B:(si + 1) * B], ktp[:])
                for h in range(HPC):
                    tp = psum.tile([B, B], BF16, tag="C", bufs=2)
                    nc.tensor.transpose(tp[:], q_t[:, h * B:(h + 1) * B], ident_sb[:])
                    dst = qT_sb[:, h * S + si * B:h * S + (si + 1) * B]
                    if h % 2 == 0:
                        nc.vector.tensor_copy(dst, tp[:])
                    else:
                        nc.scalar.copy(dst, tp[:])

                if si == 7:
                    for h in range(HPC):
                        pT = roam.tile([B, PT_COLS], BF16, tag="roam", bufs=4,
                                       name=f"pT{h}")
                        pTs.append(pT)
                for (h, kj, s1) in sched[si]:
                    _scores(h, kj, s1, chunked=True)
                    done.add((h, kj))
                qi_sched = {12: (0, 4), 13: (1, 5, 8), 14: (2, 6, 9),
                            15: (3, 7)}
                for qi in qi_sched.get(si, ()):
                    _chain_qi(qi)

            # si 0/1: hi-parts interleaved per weight batch, then lo-parts
            # paced by the wl stream, then si2/si3 at full speed
            EARLY_PE = []
            for b in range(4):
                EARLY_PE += [(1, 0, b, b + 1), (2, 0, b, b + 1)]
            for b in range(4):
                EARLY_PE += [(1, 1, b, b + 1), (2, 1, b, b + 1)]
            for b in range(4):
                EARLY_PE += [(3, 0, b, b + 1), (3, 1, b, b + 1)]
            EARLY_PE += [("fin", 0), ("fin", 1)]
            for ent in EARLY_PE:
                if ent[0] == "fin":
                    _finish_si(ent[1])
                else:
                    _qkv_part(ent[1], ent[0], ent[2], ent[3])
            for si in (2, 3):
                for term in (1, 2, 3):
                    _qkv_part(si, term, 0, 4)
                _finish_si(si)

            for si in range(4, NSB):
                t_ = work.tile([B, NCB, B], F8, tag="xh", bufs=3)
                nc.sync.dma_start(out=t_[:], in_=xh_h[si].bitcast(F8))
                early_xh[si] = t_
                t_ = work.tile([B, NCB, B], F8, tag="xl", bufs=3)
                nc.sync.dma_start(out=t_[:], in_=xl_h[si].bitcast(F8))
                early_xl[si] = t_
                t_ = work.tile([B, 512], F32, tag="cs", bufs=3)
                nc.sync.dma_start(out=t_[:], in_=cs_h[si])
                early_cs[si] = t_
                for term in (1, 2, 3):
                    _qkv_part(si, term, 0, 4)
                _finish_si(si)

            # wo reuses the wqkv slots; attnT is its own tensor so chains can
            # write it before the last qkv matmul retires
            woh_sb = bigp.tile([B, HPC, DIM], F8, tag="bigh")
            wol_sb = bigp.tile([B, HPC, DIM], F8, tag="bigl")
            for g in range(4):
                g0, g1 = g * 1024, (g + 1) * 1024
                for t in range(HPC // 2):
                    nc.sync.dma_start(out=woh_sb[:, 2 * t:2 * t + 2, g0:g1],
                                      in_=woh_h[t][:, :, g0:g1].bitcast(F8))
                for t in range(HPC // 2):
                    nc.sync.dma_start(out=wol_sb[:, 2 * t:2 * t + 2, g0:g1],
                                      in_=wol_h[t][:, :, g0:g1].bitcast(F8))

            # ===== Phase B2: PV + normalize, interleaved with wo per scol =====
            # seq0/seq1 chains (except 10/11) ran inside phase A; the
            # latency-bound leftover chains and seq2 scores/chains interleave
            # with wo(0)'s dense cp groups, covering the wo weight stream
            units = []
            for h in range(HPC):
                units.append(("sc", 12, h))
            for h in range(HPC):
                units.append(("sc", 13, h))
                units.append(("ch", 10, h))
            for h in range(HPC):
                units.append(("sc", 14, h))
                units.append(("ch", 11, h))
            for h in range(HPC):
                units.append(("sc", 15, h))
            for qi in (12, 13, 14, 15):
                for h in range(HPC):
                    units.append(("ch", qi, h))
            cps = [(s, c) for s in range(3) for c in range(NCB)]
            nu, ncp = len(units), len(cps)
            ci = 0
            RAMP = 10   # no cps among the first units: their wo weights are
                       # still streaming and a blocked cp stalls the in-order
                       # PE queue behind it
            for i, u in enumerate(units):
                if u[0] == "ch":
                    _chain_head(u[1], u[2], tp_tag="B")
                else:
                    if (u[2], u[1]) not in done:
                        _scores(u[2], u[1], 16)
                if i < RAMP:
                    continue
                take = ((i + 1 - RAMP) * ncp) // (nu - RAMP) - ((i - RAMP) * ncp) // (nu - RAMP)
                for s, c in cps[ci:ci + take]:
                    _wo(s, [c])
                ci += take
            for s, c in cps[ci:]:
                _wo(s, [c])
            _wo(3)

    nc.finalize()
    return nc


def get_program():
    global _PROGRAM
    if _PROGRAM is None:
        _PROGRAM = _build_program()
    return _PROGRAM


def make_in_maps(x, cos, sin, wq, wk, wv, wo):
    import ml_dtypes
    f8 = ml_dtypes.float8_e4m3fn

    def split8(v):
        hi = v.astype(f8)
        lo = (v - hi.astype(np.float32)).astype(f8)
        return hi, lo

    x = np.asarray(x, np.float32)
    cos = np.asarray(cos, np.float32)
    sin = np.asarray(sin, np.float32)
    wq = np.asarray(wq, np.float32)
    wk = np.asarray(wk, np.float32)
    wv = np.asarray(wv, np.float32)
    wo = np.asarray(wo, np.float32)

    # xt[si, p, cb, s] = x[si*B + s, cb*B + p]
    xt = np.ascontiguousarray(
        x.reshape(NSB, B, NCB, B).transpose(0, 3, 2, 1)
    )
    xh, xl = split8(xt)
    # cos||sin tiled 4x along channels (per-head repeat), blocked by si,
    # divided by 512 to cancel the 512*sqrt(scale) on wq/wk (leaving sqrt(scale))
    cosr = np.tile(cos / 512.0, (1, HPC)).reshape(NSB, B, 2 * HEAD_DIM)
    sinr = np.tile(sin / 512.0, (1, HPC)).reshape(NSB, B, 2 * HEAD_DIM)
    csr = np.ascontiguousarray(np.concatenate([cosr, sinr], axis=2))
    # diagonal-block causal bias in scoresT layout: allow sq >= sk
    i = np.arange(B)
    dmask = np.where(i[None, :] >= i[:, None], 0.0, -30000.0).astype(np.float32)
    ident = np.eye(B, dtype=np.float32).astype(ml_dtypes.bfloat16)

    W768 = QW + 2 * KW
    in_maps = []
    for c in range(N_CORES):
        wq_c = wq[:, c * QW:(c + 1) * QW] * F_W
        wk_c = wk[:, c * KW:(c + 1) * KW] * F_W
        wv_c = wv[:, c * KW:(c + 1) * KW] * F_W
        wqkv_c = np.concatenate([wq_c, wk_c, wv_c], axis=1)
        # blocked [b, p, u, j] = wqkv[(8b+u)*B + p, j]
        wqkv_b = np.ascontiguousarray(
            wqkv_c.reshape(NCB // 8, 8, B, W768).transpose(0, 2, 1, 3))
        whb, wlb = split8(wqkv_b)
        wo_c = wo[c * QW:(c + 1) * QW, :] * E_WO
        wo_b = np.ascontiguousarray(
            wo_c.reshape(HPC // 2, 2, B, DIM).transpose(0, 2, 1, 3))
        woh, wol = split8(wo_b)
        in_maps.append({
            "xh": xh.view(np.uint8),
            "xl": xl.view(np.uint8),
            "whb": whb.view(np.uint8),
            "wlb": wlb.view(np.uint8),
            "woh": woh.view(np.uint8),
            "wol": wol.view(np.uint8),
            "csr": csr,
            "dmask": dmask,
            "ident": ident,
        })
    return in_maps


def combine_outputs(results):
    acc = np.zeros((NCB // 4, B, 4, 4, 512), np.float32)
    for r in results:
        acc += np.asarray(r["outp"]).astype(np.float32)
    acc *= 1.0 / E_WO
    # [cpg, p, j, scol, col] -> [cpg, j, p, scol, col] -> [DIM, S] -> [S, DIM]
    full = acc.transpose(0, 2, 1, 3, 4).reshape(DIM, S)
    return np.ascontiguousarray(full.T)


def kernel(x, cos, sin, mask, wq, wk, wv, wo):
    nc = get_program()
    in_maps = make_in_maps(x, cos, sin, wq, wk, wv, wo)
    res = run_bass_kernel_spmd(nc, in_maps, core_ids=list(range(N_CORES)))
    return combine_outputs(res.results)


# revision 58
# speedup vs baseline: 1.2056x; 1.0040x over previous
"""GQA sparse attention (packed seqs + sliding window + RoPE) on 8 Trainium2 cores.

Sharding: tensor-parallel over heads. Each of the 8 cores owns 4 Q-heads and
their single shared KV-head (GQA groups stay intact): wq columns
[h*512:(h+1)*512], wk/wv columns [h*128:(h+1)*128], wo rows [h*512:(h+1)*512].
Every core computes a full [S, DIM] partial of the output projection; the host
sums the 8 partials.

The mask never reaches the device: seqlens [1024, 512, 512] with causal +
sliding-window 1024 reduce to block-causal over 128-blocks within each
sequence (the window can never truncate since max causal span == 1024), plus
a causal bias on the diagonal 128x128 blocks.

The two dense projections (qkv, wo) run as fp8-e4m3 DoubleRow matmuls
(2 contraction rows per partition, 0.5 PE cycles per output column = 4x bf16
throughput) with a 3-term residual split for accuracy:
    A @ W  ~=  A_hi @ W_hi + A_lo @ W_hi + A_hi @ W_lo
where T_hi = fp8(T), T_lo = fp8(T - T_hi). Weight scales are arranged on the
host so every fp8 tensor sits in e4m3's sweet range: wq/wk/wv carry
512*sqrt(scale), rope's cos/sin carry 1/512 (leaving q,k scaled by
sqrt(scale) so scores come out exact), v's 512*sqrt(scale) cancels against
the softmax denominator by setting the appended ones-column to that same
constant, and wo carries 32 which the host divides back out.

Per-core dataflow (attention middle stays bf16 with fp32 PSUM):
  qkv:   psum[s,768] = sum_t sum_term x?[2t:2t+2].T @ w?[2t:2t+2]   (DR fp8)
  rope:  strided DVE ops on the psum, [s,d] layout, fp32 in / bf16 out
  qT/kT: PE transposes of the roped blocks
  scores(T): psum[sk, sq_span] = kT_blk.T @ qT[h]         (block-causal spans)
  p:     exp(scores + diag_bias) -> pT buffer, bf16
  pv:    psum[sq, 129] = sum_kj pT_blk.T @ [v_blk | ones*F]
  out:   attn = pv[:, :128] * recip(pv[:, 128]); transpose; split fp8 hi/lo
  wo:    psum[c',s] = sum_t sum_term wo?[2t:2t+2].T @ attnT?[2t:2t+2] (DR fp8)
"""

import os

os.environ.setdefault("JAX_PLATFORMS", "axon")

import numpy as np

import concourse.bass as bass
import concourse.mybir as mybir
import concourse.tile as tile
from concourse import bacc
from concourse.bass_utils import run_bass_kernel_spmd

# ---- problem constants (hardcoded per harness contract) ----
DIM = 4096
N_HEADS = 32
N_KV_HEADS = 8
HEAD_DIM = 128
SEQLENS = [1024, 512, 512]
S = 2048
N_CORES = 8
HPC = N_HEADS // N_CORES          # q heads per core = 4
QW = HPC * HEAD_DIM               # per-core q width = 512
KW = HEAD_DIM                     # per-core k/v width = 128
B = 128                           # block size
NSB = S // B                      # 16 seq blocks
NCB = DIM // B                    # 32 contraction blocks
SEQ_BLOCKS = []                   # [(start_blk, end_blk)] per packed sequence
_b = 0
for _l in SEQLENS:
    SEQ_BLOCKS.append((_b, _b + _l // B))
    _b += _l // B

# fp8 scale plumbing (see module docstring)
F_W = 512.0 * (HEAD_DIM ** -0.25)   # wq/wk/wv host scale; ones column value
E_WO = 32.0                         # wo host scale, divided out on host

# pT buffer layout: for each kj, columns [offs[kj] : offs[kj]+span(kj)) hold
# p.T for queries sq in [kj*B, seq_end)
_SPANS = {}
_OFFS = {}
_off = 0
for _s0, _s1 in SEQ_BLOCKS:
    for _kj in range(_s0, _s1):
        _SPANS[_kj] = (_s1 - _kj) * B
        _OFFS[_kj] = _off
        _off += _SPANS[_kj]
PT_COLS = _off                    # 7168

F32 = mybir.dt.float32
BF16 = mybir.dt.bfloat16
F8 = mybir.dt.float8e4
DR = mybir.MatmulPerfMode.DoubleRow

_PROGRAM = None


def _build_program():
    nc = bacc.Bacc(trn_type="TRN2")

    xh_h = nc.declare_dram_parameter("xh", [NSB, B, NCB, B], mybir.dt.uint8, isOutput=False)
    xl_h = nc.declare_dram_parameter("xl", [NSB, B, NCB, B], mybir.dt.uint8, isOutput=False)
    whb_h = nc.declare_dram_parameter("whb", [NCB // 8, B, 8, QW + 2 * KW], mybir.dt.uint8, isOutput=False)
    wlb_h = nc.declare_dram_parameter("wlb", [NCB // 8, B, 8, QW + 2 * KW], mybir.dt.uint8, isOutput=False)
    woh_h = nc.declare_dram_parameter("woh", [HPC // 2, B, 2, DIM], mybir.dt.uint8, isOutput=False)
    wol_h = nc.declare_dram_parameter("wol", [HPC // 2, B, 2, DIM], mybir.dt.uint8, isOutput=False)
    cs_h = nc.declare_dram_parameter("csr", [NSB, B, 4 * HEAD_DIM], F32, isOutput=False)
    dmask_h = nc.declare_dram_parameter("dmask", [B, B], F32, isOutput=False)
    ident_h = nc.declare_dram_parameter("ident", [B, B], BF16, isOutput=False)
    # out layout [cpg, p, j, scol, col]: a [B, 4, 512] SBUF tile DMAs to
    # outp[cpg, :, :, scol, :] with matching iteration order (host reassembles)
    out_h = nc.declare_dram_parameter("outp", [NCB // 4, B, 4, 4, 512], BF16, isOutput=True)

    W768 = QW + 2 * KW  # 768
    Exp = mybir.ActivationFunctionType.Exp

    with tile.TileContext(nc) as tc:
        with (
            tc.tile_pool(name="consts", bufs=1) as cpool,
            tc.tile_pool(name="big", bufs=1) as bigp,
            tc.tile_pool(name="persist", bufs=1) as pers,
            tc.tile_pool(name="roam", bufs=4) as roam,
            tc.tile_pool(name="work", bufs=3) as work,
            tc.tile_pool(name="psum", bufs=2, space="PSUM") as psum,
        ):
            # startup is DMA-bound: the stream order below is tuned so PE
            # always has runnable qkv work (see EARLY_DMA / EARLY_PE)
            wh_sb = bigp.tile([B, NCB, W768], F8, tag="bigh")
            wl_sb = bigp.tile([B, NCB, W768], F8, tag="bigl")
            early_xh = {}
            early_xl = {}
            early_cs = {}

            EARLY_DMA = [
                ("xhA", 0), ("wh", 0), ("xlA", 0), ("xhB", 0), ("wh", 1),
                ("xlB", 0), ("wh", 2), ("wh", 3), ("xh", 1), ("xl", 1),
                ("const",), ("cs", 0), ("cs", 1), ("wl", 0), ("wl", 1),
                ("wl", 2), ("wl", 3), ("xh", 2), ("xl", 2), ("cs", 2),
                ("xh", 3), ("xl", 3), ("cs", 3),
            ]
            ident_sb = cpool.tile([B, B], BF16)
            dmask_sb = cpool.tile([B, B], F32)
            for ent in EARLY_DMA:
                kind = ent[0]
                if kind in ("xhA", "xlA", "xhB", "xlB"):
                    si_, hl = ent[1], kind[1]
                    dram = xh_h if hl == "h" else xl_h
                    store = early_xh if hl == "h" else early_xl
                    if kind[2] == "A":
                        t_ = work.tile([B, NCB, B], F8, tag="x" + hl, bufs=3,
                                       name=f"x{hl}p{si_}")
                        store[si_] = t_
                        nc.sync.dma_start(out=t_[:, 0:16, :],
                                          in_=dram[si_][:, 0:16, :].bitcast(F8))
                    else:
                        t_ = store[si_]
                        nc.sync.dma_start(out=t_[:, 16:32, :],
                                          in_=dram[si_][:, 16:32, :].bitcast(F8))
                elif kind == "xh":
                    t_ = work.tile([B, NCB, B], F8, tag="xh", bufs=3,
                                   name=f"xhp{ent[1]}")
                    nc.sync.dma_start(out=t_[:], in_=xh_h[ent[1]].bitcast(F8))
                    early_xh[ent[1]] = t_
                elif kind == "xl":
                    t_ = work.tile([B, NCB, B], F8, tag="xl", bufs=3,
                                   name=f"xlp{ent[1]}")
                    nc.sync.dma_start(out=t_[:], in_=xl_h[ent[1]].bitcast(F8))
                    early_xl[ent[1]] = t_
                elif kind == "wh":
                    b = ent[1]
                    nc.sync.dma_start(out=wh_sb[:, 8 * b:8 * b + 8, :],
                                      in_=whb_h[b].bitcast(F8))
                elif kind == "wl":
                    b = ent[1]
                    nc.sync.dma_start(out=wl_sb[:, 8 * b:8 * b + 8, :],
                                      in_=wlb_h[b].bitcast(F8))
                elif kind == "cs":
                    t_ = work.tile([B, 512], F32, tag="cs", bufs=3,
                                   name=f"csp{ent[1]}")
                    nc.sync.dma_start(out=t_[:], in_=cs_h[ent[1]])
                    early_cs[ent[1]] = t_
                else:
                    nc.sync.dma_start(out=ident_sb[:], in_=ident_h[:])
                    nc.sync.dma_start(out=dmask_sb[:], in_=dmask_h[:])

            attnT_hi = pers.tile([B, HPC, S], F8)     # [d, head, seq]
            attnT_lo = pers.tile([B, HPC, S], F8)
            qT_sb = pers.tile([B, HPC * S], BF16)     # per head h: cols [h*S, (h+1)*S)
            kT_sb = pers.tile([B, S], BF16)
            vaug_sb = pers.tile([B, NSB * 129], BF16)  # per kj: [v_blk | ones*F_W]

            # scores + exp for one (head, kj) block-row
            pTs = []

            def _scores(h, kj, s1, chunked=False):
                pT = pTs[h]
                span = (s1 - kj) * B
                if chunked:
                    # phase-A-overlapped variant: 512-col chunks through tag B
                    # (PV's tag, idle during phase A) so the qkv psum pipeline
                    # in tag A is never paced by exp latency
                    for part in range(0, span, 512):
                        n = min(512, span - part)
                        ps_c = psum.tile([B, 512], F32, tag="B", bufs=2,
                                         name="ps_c")
                        nc.tensor.matmul(
                            ps_c[:, 0:n],
                            kT_sb[:, kj * B:(kj + 1) * B],
                            qT_sb[:, h * S + kj * B + part:
                                  h * S + kj * B + part + n],
                            start=True, stop=True,
                        )
                        if part == 0:
                            nc.vector.tensor_add(
                                ps_c[:, 0:B], ps_c[:, 0:B], dmask_sb[:]
                            )
                        nc.scalar.activation(
                            pT[:, _OFFS[kj] + part:_OFFS[kj] + part + n],
                            ps_c[:, 0:n], Exp
                        )
                    return
                ps_sc = psum.tile([B, 1024], F32, tag="A", bufs=2, name="ps_sc")
                for part in range(0, span, 512):
                    n = min(512, span - part)
                    nc.tensor.matmul(
                        ps_sc[:, part:part + n],
                        kT_sb[:, kj * B:(kj + 1) * B],
                        qT_sb[:, h * S + kj * B + part:
                              h * S + kj * B + part + n],
                        start=True, stop=True,
                    )
                # causal bias on the diagonal block
                nc.vector.tensor_add(ps_sc[:, 0:B], ps_sc[:, 0:B], dmask_sb[:])
                nc.scalar.activation(
                    pT[:, _OFFS[kj]:_OFFS[kj] + span], ps_sc[:, 0:span], Exp
                )

            # seq0/seq1 score groups overlap the back half of the qkv phase
            # (their qT/kT inputs are complete by then); seq2 runs after
            chains_done = set()
            sched = {si: [] for si in range(NSB)}
            for kj in range(0, 8):
                sched[8 + kj] = [(h, kj, 8) for h in range(HPC)]
            for kj, si in ((8, 12), (9, 13), (10, 14), (11, 15)):
                sched[si] += [(h, kj, 12) for h in range(HPC)]
            done = set()

            def _chain_head(qi, h, tp_tag="C"):
                    s0, s1 = next(b for b in SEQ_BLOCKS if b[0] <= qi < b[1])
                    if True:
                        pT = pTs[h]
                        ps_pv = psum.tile([B, 129], F32, tag="B", bufs=2)
                        for kj in range(s0, qi + 1):
                            lhsT = pT[:, _OFFS[kj] + (qi - kj) * B:
                                      _OFFS[kj] + (qi - kj + 1) * B]
                            nc.tensor.matmul(
                                ps_pv[:], lhsT,
                                vaug_sb[:, kj * 129:(kj + 1) * 129],
                                start=(kj == s0), stop=(kj == qi),
                            )
                        rc = work.tile([B, 1], F32, tag="rc", bufs=8)
                        nc.vector.reciprocal(rc[:], ps_pv[:, 128:129])
                        at = work.tile([B, B], BF16, tag="at", bufs=6)
                        nc.vector.tensor_scalar_mul(at[:], ps_pv[:, 0:B], rc[:])
                        tp = psum.tile([B, B], BF16, tag=tp_tag, bufs=2)
                        nc.tensor.transpose(tp[:], at[:], ident_sb[:])
                        hi_dst = attnT_hi[:, h, qi * B:(qi + 1) * B]
                        lo_dst = attnT_lo[:, h, qi * B:(qi + 1) * B]
                        nc.scalar.copy(hi_dst, tp[:])
                        nc.vector.tensor_sub(lo_dst, tp[:], hi_dst)

            ot_cur = {}

            def _chain_qi(qi, tp_tag="C"):
                chains_done.add(qi)
                for h in range(HPC):
                    _chain_head(qi, h, tp_tag)

            def _wo(scol, cps=None):
                for cp in (range(NCB) if cps is None else cps):
                    # alternate accumulators across tags C/A (the qkv psum in
                    # A is released by the stage-copy; B stays with chains)
                    if cp % 2 == 1:
                        psoF = psum.tile([B, 512], F32, tag="A", bufs=2,
                                         name="psoA")
                    else:
                        psoF = psum.tile([B, 512], F32, tag="C", bufs=2,
                                         name="psoC")
                    for sub in range(2):
                        pso = psoF[:, sub * 256:(sub + 1) * 256]
                        c0 = scol * 512 + sub * 256
                        nmm = 0
                        for t in range(HPC // 2):
                            for wa, aa in ((woh_sb, attnT_hi),
                                           (wol_sb, attnT_hi),
                                           (woh_sb, attnT_lo)):
                                nc.tensor.matmul(
                                    pso,
                                    wa[:, 2 * t:2 * t + 2, cp * B:(cp + 1) * B],
                                    aa[:, 2 * t:2 * t + 2, c0:c0 + 256],
                                    start=(nmm == 0), stop=(nmm == 5),
                                    perf_mode=DR,
                                )
                                nmm += 1
                    j = cp % 4
                    if scol == 3 and cp >= 28:
                        if j in (0, 2):
                            ot_cur[scol] = work.tile([B, 2, 512], BF16,
                                                     tag="ot1", bufs=2,
                                                     name="ott")
                        ot = ot_cur[scol]
                        if cp % 2 == 0:
                            nc.scalar.copy(ot[:, j % 2, :], psoF[:])
                        else:
                            nc.vector.tensor_copy(ot[:, j % 2, :], psoF[:])
                        if j in (1, 3):
                            nc.sync.dma_start(
                                out=out_h[cp // 4][:, j - 1:j + 1, scol, :],
                                in_=ot[:],
                            )
                        continue
                    if j == 0:
                        ot_cur[scol] = work.tile([B, 4, 512], BF16, tag="ot",
                                                 bufs=2, name="otb")
                    ot = ot_cur[scol]
                    if cp % 2 == 0:
                        nc.scalar.copy(ot[:, j, :], psoF[:])
                    else:
                        nc.vector.tensor_copy(ot[:, j, :], psoF[:])
                    if j == 3:
                        nc.sync.dma_start(
                            out=out_h[cp // 4][:, :, scol, :],
                            in_=ot[:],
                        )

            # =========== Phase A: qkv projection + rope + transposes ===========
            # Per si, the three residual terms split into a hi-part (terms
            # needing only the hi weights: xh@wh, xl@wh) and a lo-part
            # (xh@wl); the psum group opens at the first hi matmul and closes
            # at the last lo matmul. For si 0-3 the parts are emitted in a
            # hand-tuned order interleaved with the DMA stream; si 4-15 are
            # PE-bound and run parts back to back.
            ps_by_si = {}
            CHUNKS = ((512, 768), (0, 256), (256, 512))

            def _qkv_part(si, term, b0, b1):
                if si not in ps_by_si:
                    ps_by_si[si] = psum.tile([B, W768], F32, tag="A", bufs=2,
                                             name=f"ps{si % 2}")
                ps = ps_by_si[si]
                xa = early_xh[si] if term != 2 else early_xl[si]
                wa = wh_sb if term != 3 else wl_sb
                # chunks q0 [0:256] and q1 [256:512] share one 2KB psum
                # zero region: only q0's first matmul carries start=True (its
                # pending-zero covers q1's bytes); giving q1 its own start
                # while q0's group is still open would wipe q0's partials
                for c0, c1 in CHUNKS:
                    for t in range(4 * b0, 4 * b1):
                        nc.tensor.matmul(
                            ps[:, c0:c1],
                            xa[:, 2 * t:2 * t + 2, :],
                            wa[:, 2 * t:2 * t + 2, c0:c1],
                            start=(term == 1 and t == 0 and c0 != 256),
                            stop=(term == 3 and t == NCB // 2 - 1),
                            perf_mode=DR,
                            skip_group_check=True,
                        )

            def _finish_si(si):
                psum_ps = ps_by_si.pop(si)
                cs_t = early_cs.pop(si)
                cs = cs_t[:, 0:256]
                sn = cs_t[:, 256:512]

                # stage the qkv psum to SBUF in one fast Act copy: releases
                # the psum buffer for si+2 immediately, and the strided rope
                # reads below hit SBUF (58-cycle access) instead of PSUM (120)
                ps = work.tile([B, W768], F32, tag="qkvs", bufs=2)
                nc.scalar.copy(ps[:], psum_ps[:])

                # rope on k first (kT feeds the phase-A score groups)
                k_t = work.tile([B, KW], BF16, tag="k", bufs=3)
                ke, ko = ps[:, 512:640:2], ps[:, 513:640:2]
                c64, s64 = cs_t[:, 0:64], cs_t[:, 256:320]
                u1 = work.tile([B, 64], F32, tag="u1", bufs=2)
                u2 = work.tile([B, 64], F32, tag="u2", bufs=2)
                u3 = work.tile([B, 64], F32, tag="u1", bufs=2, name="u3")
                u4 = work.tile([B, 64], F32, tag="u2", bufs=2, name="u4")
                nc.vector.tensor_mul(u1[:], ke, c64)
                nc.vector.tensor_mul(u2[:], ko, s64)
                nc.vector.tensor_sub(k_t[:, 0:KW:2], u1[:], u2[:])
                nc.vector.tensor_mul(u3[:], ke, s64)
                nc.vector.tensor_mul(u4[:], ko, c64)
                nc.vector.tensor_add(k_t[:, 1:KW:2], u3[:], u4[:])

                # v block + ones column (ones = F_W cancels v's host scale)
                nc.scalar.copy(vaug_sb[:, si * 129:si * 129 + 128], ps[:, 640:768])
                nc.vector.memset(vaug_sb[:, si * 129 + 128:si * 129 + 129], F_W)

                # rope on q: [s, d] layout, channels interleaved (even, odd)
                q_t = work.tile([B, QW], BF16, tag="q", bufs=3)
                qe, qo = ps[:, 0:QW:2], ps[:, 1:QW:2]
                t1 = work.tile([B, 256], F32, tag="t1", bufs=2)
                t2 = work.tile([B, 256], F32, tag="t2", bufs=2)
                t3 = work.tile([B, 256], F32, tag="t1", bufs=2, name="t3")
                t4 = work.tile([B, 256], F32, tag="t2", bufs=2, name="t4")
                # even channels on DVE, odd channels concurrently on the
                # otherwise-idle GPSIMD engine (all SBUF-to-SBUF post-stage)
                nc.vector.tensor_mul(t1[:], qe, cs)
                nc.vector.tensor_mul(t2[:], qo, sn)
                nc.vector.tensor_sub(q_t[:, 0:QW:2], t1[:], t2[:])
                nc.gpsimd.tensor_mul(t3[:], qe, sn)
                nc.gpsimd.tensor_mul(t4[:], qo, cs)
                nc.gpsimd.tensor_add(q_t[:, 1:QW:2], t3[:], t4[:])

                # transposes: k first (feeds scores), then q (4 blocks)
                ktp = psum.tile([B, B], BF16, tag="C", bufs=2)
                nc.tensor.transpose(ktp[:], k_t[:], ident_sb[:])
                nc.vector.tensor_copy(kT_sb[:, si * B:(si + 1) * B], ktp[:])
                for h in range(HPC):
                    tp = psum.tile([B, B], BF16, tag="C", bufs=2)
                    nc.tensor.transpose(tp[:], q_t[:, h * B:(h + 1) * B], ident_sb[:])
                    dst = qT_sb[:, h * S + si * B:h * S + (si + 1) * B]
                    if h % 2 == 0:
                        nc.vector.tensor_copy(dst, tp[:])
                    else:
                        nc.scalar.copy(dst, tp[:])

                if si == 7:
                    for h in range(HPC):
                        pT = roam.tile([B, PT_COLS], BF16, tag="roam", bufs=4,
                                       name=f"pT{h}")
                        pTs.append(pT)
                for (h, kj, s1) in sched[si]:
                    _scores(h, kj, s1, chunked=True)
                    done.add((h, kj))
                qi_sched = {12: (0, 4), 13: (1, 5, 8), 14: (2, 6, 9),
                            15: (3, 7)}
                for qi in qi_sched.get(si, ()):
                    _chain_qi(qi)

            # si 0/1: hi-parts interleaved per weight batch, then lo-parts
            # paced by the wl stream, then si2/si3 at full speed
            EARLY_PE = []
            for b in range(4):
                EARLY_PE += [(1, 0, b, b + 1), (2, 0, b, b + 1)]
            for b in range(4):
                EARLY_PE += [(1, 1, b, b + 1), (2, 1, b, b + 1)]
            for b in range(3):
                EARLY_PE += [(3, 0, b, b + 1), (3, 1, b, b + 1)]
            EARLY_PE += [(3, 0, 3, 4), ("fin", 0), (3, 1, 3, 4), ("fin", 1)]
            for ent in EARLY_PE:
                if ent[0] == "fin":
                    _finish_si(ent[1])
                else:
                    _qkv_part(ent[1], ent[0], ent[2], ent[3])
            for si in (2, 3):
                for term in (1, 2, 3):
                    _qkv_part(si, term, 0, 4)
                _finish_si(si)

            for si in range(4, NSB):
                t_ = work.tile([B, NCB, B], F8, tag="xh", bufs=3)
                nc.sync.dma_start(out=t_[:], in_=xh_h[si].bitcast(F8))
                early_xh[si] = t_
                t_ = work.tile([B, NCB, B], F8, tag="xl", bufs=3)
                nc.sync.dma_start(out=t_[:], in_=xl_h[si].bitcast(F8))
                early_xl[si] = t_
                t_ = work.tile([B, 512], F32, tag="cs", bufs=3)
                nc.sync.dma_start(out=t_[:], in_=cs_h[si])
                early_cs[si] = t_
                for term in (1, 2, 3):
                    _qkv_part(si, term, 0, 4)
                _finish_si(si)

            # wo reuses the wqkv slots; attnT is its own tensor so chains can
            # write it before the last qkv matmul retires
            woh_sb = bigp.tile([B, HPC, DIM], F8, tag="bigh")
            wol_sb = bigp.tile([B, HPC, DIM], F8, tag="bigl")
            for g in range(4):
                g0, g1 = g * 1024, (g + 1) * 1024
                for t in range(HPC // 2):
                    nc.sync.dma_start(out=woh_sb[:, 2 * t:2 * t + 2, g0:g1],
                                      in_=woh_h[t][:, :, g0:g1].bitcast(F8))
                for t in range(HPC // 2):
                    nc.sync.dma_start(out=wol_sb[:, 2 * t:2 * t + 2, g0:g1],
                                      in_=wol_h[t][:, :, g0:g1].bitcast(F8))

            # ===== Phase B2: PV + normalize, interleaved with wo per scol =====
            # seq0/seq1 chains (except 10/11) ran inside phase A; the
            # latency-bound leftover chains and seq2 scores/chains interleave
            # with wo(0)'s dense cp groups, covering the wo weight stream
            units = []
            for h in range(HPC):
                units.append(("sc", 12, h))
            for h in range(HPC):
                units.append(("sc", 13, h))
                units.append(("ch", 10, h))
            for h in range(HPC):
                units.append(("sc", 14, h))
                units.append(("ch", 11, h))
            for h in range(HPC):
                units.append(("sc", 15, h))
            for qi in (12, 13, 14, 15):
                for h in range(HPC):
                    units.append(("ch", qi, h))
            cps = [(s, c) for s in range(3) for c in range(NCB)]
            nu, ncp = len(units), len(cps)
            ci = 0
            RAMP = 10   # no cps among the first units: their wo weights are
                       # still streaming and a blocked cp stalls the in-order
                       # PE queue behind it
            for i, u in enumerate(units):
                if u[0] == "ch":
                    _chain_head(u[1], u[2], tp_tag="B")
                else:
                    if (u[2], u[1]) not in done:
                        _scores(u[2], u[1], 16)
                if i < RAMP:
                    continue
                take = ((i + 1 - RAMP) * ncp) // (nu - RAMP) - ((i - RAMP) * ncp) // (nu - RAMP)
                for s, c in cps[ci:ci + take]:
                    _wo(s, [c])
                ci += take
            for s, c in cps[ci:]:
                _wo(s, [c])
            _wo(3)

    nc.finalize()
    return nc


def get_program():
    global _PROGRAM
    if _PROGRAM is None:
        _PROGRAM = _build_program()
    return _PROGRAM


def make_in_maps(x, cos, sin, wq, wk, wv, wo):
    import ml_dtypes
    f8 = ml_dtypes.float8_e4m3fn

    def split8(v):
        hi = v.astype(f8)
        lo = (v - hi.astype(np.float32)).astype(f8)
        return hi, lo

    x = np.asarray(x, np.float32)
    cos = np.asarray(cos, np.float32)
    sin = np.asarray(sin, np.float32)
    wq = np.asarray(wq, np.float32)
    wk = np.asarray(wk, np.float32)
    wv = np.asarray(wv, np.float32)
    wo = np.asarray(wo, np.float32)

    # xt[si, p, cb, s] = x[si*B + s, cb*B + p]
    xt = np.ascontiguousarray(
        x.reshape(NSB, B, NCB, B).transpose(0, 3, 2, 1)
    )
    xh, xl = split8(xt)
    # cos||sin tiled 4x along channels (per-head repeat), blocked by si,
    # divided by 512 to cancel the 512*sqrt(scale) on wq/wk (leaving sqrt(scale))
    cosr = np.tile(cos / 512.0, (1, HPC)).reshape(NSB, B, 2 * HEAD_DIM)
    sinr = np.tile(sin / 512.0, (1, HPC)).reshape(NSB, B, 2 * HEAD_DIM)
    csr = np.ascontiguousarray(np.concatenate([cosr, sinr], axis=2))
    # diagonal-block causal bias in scoresT layout: allow sq >= sk
    i = np.arange(B)
    dmask = np.where(i[None, :] >= i[:, None], 0.0, -30000.0).astype(np.float32)
    ident = np.eye(B, dtype=np.float32).astype(ml_dtypes.bfloat16)

    W768 = QW + 2 * KW
    in_maps = []
    for c in range(N_CORES):
        wq_c = wq[:, c * QW:(c + 1) * QW] * F_W
        wk_c = wk[:, c * KW:(c + 1) * KW] * F_W
        wv_c = wv[:, c * KW:(c + 1) * KW] * F_W
        wqkv_c = np.concatenate([wq_c, wk_c, wv_c], axis=1)
        # blocked [b, p, u, j] = wqkv[(8b+u)*B + p, j]
        wqkv_b = np.ascontiguousarray(
            wqkv_c.reshape(NCB // 8, 8, B, W768).transpose(0, 2, 1, 3))
        whb, wlb = split8(wqkv_b)
        wo_c = wo[c * QW:(c + 1) * QW, :] * E_WO
        wo_b = np.ascontiguousarray(
            wo_c.reshape(HPC // 2, 2, B, DIM).transpose(0, 2, 1, 3))
        woh, wol = split8(wo_b)
        in_maps.append({
            "xh": xh.view(np.uint8),
            "xl": xl.view(np.uint8),
            "whb": whb.view(np.uint8),
            "wlb": wlb.view(np.uint8),
            "woh": woh.view(np.uint8),
            "wol": wol.view(np.uint8),
            "csr": csr,
            "dmask": dmask,
            "ident": ident,
        })
    return in_maps


def combine_outputs(results):
    acc = np.zeros((NCB // 4, B, 4, 4, 512), np.float32)
    for r in results:
        acc += np.asarray(r["outp"]).astype(np.float32)
    acc *= 1.0 / E_WO
    # [cpg, p, j, scol, col] -> [cpg, j, p, scol, col] -> [DIM, S] -> [S, DIM]
    full = acc.transpose(0, 2, 1, 3, 4).reshape(DIM, S)
    return np.ascontiguousarray(full.T)


def kernel(x, cos, sin, mask, wq, wk, wv, wo):
    nc = get_program()
    in_maps = make_in_maps(x, cos, sin, wq, wk, wv, wo)
    res = run_bass_kernel_spmd(nc, in_maps, core_ids=list(range(N_CORES)))
    return combine_outputs(res.results)


# revision 65
# speedup vs baseline: 1.2087x; 1.0026x over previous
"""GQA sparse attention (packed seqs + sliding window + RoPE) on 8 Trainium2 cores.

Sharding: tensor-parallel over heads. Each of the 8 cores owns 4 Q-heads and
their single shared KV-head (GQA groups stay intact): wq columns
[h*512:(h+1)*512], wk/wv columns [h*128:(h+1)*128], wo rows [h*512:(h+1)*512].
Every core computes a full [S, DIM] partial of the output projection; the host
sums the 8 partials.

The mask never reaches the device: seqlens [1024, 512, 512] with causal +
sliding-window 1024 reduce to block-causal over 128-blocks within each
sequence (the window can never truncate since max causal span == 1024), plus
a causal bias on the diagonal 128x128 blocks.

The two dense projections (qkv, wo) run as fp8-e4m3 DoubleRow matmuls
(2 contraction rows per partition, 0.5 PE cycles per output column = 4x bf16
throughput) with a 3-term residual split for accuracy:
    A @ W  ~=  A_hi @ W_hi + A_lo @ W_hi + A_hi @ W_lo
where T_hi = fp8(T), T_lo = fp8(T - T_hi). Weight scales are arranged on the
host so every fp8 tensor sits in e4m3's sweet range: wq/wk/wv carry
512*sqrt(scale), rope's cos/sin carry 1/512 (leaving q,k scaled by
sqrt(scale) so scores come out exact), v's 512*sqrt(scale) cancels against
the softmax denominator by setting the appended ones-column to that same
constant, and wo carries 32 which the host divides back out.

Per-core dataflow (attention middle stays bf16 with fp32 PSUM):
  qkv:   psum[s,768] = sum_t sum_term x?[2t:2t+2].T @ w?[2t:2t+2]   (DR fp8)
  rope:  strided DVE ops on the psum, [s,d] layout, fp32 in / bf16 out
  qT/kT: PE transposes of the roped blocks
  scores(T): psum[sk, sq_span] = kT_blk.T @ qT[h]         (block-causal spans)
  p:     exp(scores + diag_bias) -> pT buffer, bf16
  pv:    psum[sq, 129] = sum_kj pT_blk.T @ [v_blk | ones*F]
  out:   attn = pv[:, :128] * recip(pv[:, 128]); transpose; split fp8 hi/lo
  wo:    psum[c',s] = sum_t sum_term wo?[2t:2t+2].T @ attnT?[2t:2t+2] (DR fp8)
"""

import os

os.environ.setdefault("JAX_PLATFORMS", "axon")

import numpy as np

import concourse.bass as bass
import concourse.mybir as mybir
import concourse.tile as tile
from concourse import bacc
from concourse.bass_utils import run_bass_kernel_spmd

# ---- problem constants (hardcoded per harness contract) ----
DIM = 4096
N_HEADS = 32
N_KV_HEADS = 8
HEAD_DIM = 128
SEQLENS = [1024, 512, 512]
S = 2048
N_CORES = 8
HPC = N_HEADS // N_CORES          # q heads per core = 4
QW = HPC * HEAD_DIM               # per-core q width = 512
KW = HEAD_DIM                     # per-core k/v width = 128
B = 128                           # block size
NSB = S // B                      # 16 seq blocks
NCB = DIM // B                    # 32 contraction blocks
SEQ_BLOCKS = []                   # [(start_blk, end_blk)] per packed sequence
_b = 0
for _l in SEQLENS:
    SEQ_BLOCKS.append((_b, _b + _l // B))
    _b += _l // B

# fp8 scale plumbing (see module docstring)
F_W = 512.0 * (HEAD_DIM ** -0.25)   # wq/wk/wv host scale; ones column value
E_WO = 32.0                         # wo host scale, divided out on host

# pT buffer layout: for each kj, columns [offs[kj] : offs[kj]+span(kj)) hold
# p.T for queries sq in [kj*B, seq_end)
_SPANS = {}
_OFFS = {}
_off = 0
for _s0, _s1 in SEQ_BLOCKS:
    for _kj in range(_s0, _s1):
        _SPANS[_kj] = (_s1 - _kj) * B
        _OFFS[_kj] = _off
        _off += _SPANS[_kj]
PT_COLS = _off                    # 7168

F32 = mybir.dt.float32
BF16 = mybir.dt.bfloat16
F8 = mybir.dt.float8e4
DR = mybir.MatmulPerfMode.DoubleRow

_PROGRAM = None


def _build_program():
    nc = bacc.Bacc(trn_type="TRN2")

    xh_h = nc.declare_dram_parameter("xh", [NSB, B, NCB, B], mybir.dt.uint8, isOutput=False)
    xl_h = nc.declare_dram_parameter("xl", [NSB, B, NCB, B], mybir.dt.uint8, isOutput=False)
    whb_h = nc.declare_dram_parameter("whb", [NCB // 8, B, 8, QW + 2 * KW], mybir.dt.uint8, isOutput=False)
    wlb_h = nc.declare_dram_parameter("wlb", [NCB // 8, B, 8, QW + 2 * KW], mybir.dt.uint8, isOutput=False)
    woh_h = nc.declare_dram_parameter("woh", [HPC // 2, B, 2, DIM], mybir.dt.uint8, isOutput=False)
    wol_h = nc.declare_dram_parameter("wol", [HPC // 2, B, 2, DIM], mybir.dt.uint8, isOutput=False)
    cs_h = nc.declare_dram_parameter("csr", [NSB, B, 4 * HEAD_DIM], BF16, isOutput=False)
    dmask_h = nc.declare_dram_parameter("dmask", [B, B], F32, isOutput=False)
    ident_h = nc.declare_dram_parameter("ident", [B, B], BF16, isOutput=False)
    # out layout [cpg, p, j, scol, col]: a [B, 4, 512] SBUF tile DMAs to
    # outp[cpg, :, :, scol, :] with matching iteration order (host reassembles)
    out_h = nc.declare_dram_parameter("outp", [NCB // 4, B, 4, 4, 512], BF16, isOutput=True)

    W768 = QW + 2 * KW  # 768
    Exp = mybir.ActivationFunctionType.Exp

    with tile.TileContext(nc) as tc:
        with (
            tc.tile_pool(name="consts", bufs=1) as cpool,
            tc.tile_pool(name="big", bufs=1) as bigp,
            tc.tile_pool(name="persist", bufs=1) as pers,
            tc.tile_pool(name="roam", bufs=4) as roam,
            tc.tile_pool(name="work", bufs=3) as work,
            tc.tile_pool(name="psum", bufs=2, space="PSUM") as psum,
        ):
            # startup is DMA-bound: the stream order below is tuned so PE
            # always has runnable qkv work (see EARLY_DMA / EARLY_PE)
            wh_sb = bigp.tile([B, NCB, W768], F8, tag="bigh")
            wl_sb = bigp.tile([B, NCB, W768], F8, tag="bigl")
            early_xh = {}
            early_xl = {}
            early_cs = {}

            EARLY_DMA = [
                ("xhA", 0), ("wh", 0), ("xlA", 0), ("xhB", 0), ("wh", 1),
                ("xlB", 0), ("wh", 2), ("wh", 3), ("xh", 1), ("xl", 1),
                ("const",), ("cs", 0), ("cs", 1), ("wl", 0), ("wl", 1),
                ("wl", 2), ("wl", 3), ("xh", 2), ("xl", 2), ("cs", 2),
                ("xh", 3), ("xl", 3), ("cs", 3),
            ]
            ident_sb = cpool.tile([B, B], BF16)
            dmask_sb = cpool.tile([B, B], F32)
            for ent in EARLY_DMA:
                kind = ent[0]
                if kind in ("xhA", "xlA", "xhB", "xlB"):
                    si_, hl = ent[1], kind[1]
                    dram = xh_h if hl == "h" else xl_h
                    store = early_xh if hl == "h" else early_xl
                    if kind[2] == "A":
                        t_ = work.tile([B, NCB, B], F8, tag="x" + hl, bufs=3,
                                       name=f"x{hl}p{si_}")
                        store[si_] = t_
                        nc.sync.dma_start(out=t_[:, 0:16, :],
                                          in_=dram[si_][:, 0:16, :].bitcast(F8))
                    else:
                        t_ = store[si_]
                        nc.sync.dma_start(out=t_[:, 16:32, :],
                                          in_=dram[si_][:, 16:32, :].bitcast(F8))
                elif kind == "xh":
                    t_ = work.tile([B, NCB, B], F8, tag="xh", bufs=3,
                                   name=f"xhp{ent[1]}")
                    nc.sync.dma_start(out=t_[:], in_=xh_h[ent[1]].bitcast(F8))
                    early_xh[ent[1]] = t_
                elif kind == "xl":
                    t_ = work.tile([B, NCB, B], F8, tag="xl", bufs=3,
                                   name=f"xlp{ent[1]}")
                    nc.sync.dma_start(out=t_[:], in_=xl_h[ent[1]].bitcast(F8))
                    early_xl[ent[1]] = t_
                elif kind == "wh":
                    b = ent[1]
                    nc.sync.dma_start(out=wh_sb[:, 8 * b:8 * b + 8, :],
                                      in_=whb_h[b].bitcast(F8))
                elif kind == "wl":
                    b = ent[1]
                    nc.sync.dma_start(out=wl_sb[:, 8 * b:8 * b + 8, :],
                                      in_=wlb_h[b].bitcast(F8))
                elif kind == "cs":
                    t_ = work.tile([B, 512], BF16, tag="cs", bufs=3,
                                   name=f"csp{ent[1]}")
                    nc.sync.dma_start(out=t_[:], in_=cs_h[ent[1]])
                    early_cs[ent[1]] = t_
                else:
                    nc.sync.dma_start(out=ident_sb[:], in_=ident_h[:])
                    nc.sync.dma_start(out=dmask_sb[:], in_=dmask_h[:])

            attnT_hi = pers.tile([B, HPC, S], F8)     # [d, head, seq]
            attnT_lo = pers.tile([B, HPC, S], F8)
            qT_sb = pers.tile([B, HPC * S], BF16)     # per head h: cols [h*S, (h+1)*S)
            kT_sb = pers.tile([B, S], BF16)
            vaug_sb = pers.tile([B, NSB * 129], BF16)  # per kj: [v_blk | ones*F_W]

            # scores + exp for one (head, kj) block-row
            pTs = []

            def _scores(h, kj, s1, chunked=False):
                pT = pTs[h]
                span = (s1 - kj) * B
                if chunked:
                    # phase-A-overlapped variant: 512-col chunks through tag B
                    # (PV's tag, idle during phase A) so the qkv psum pipeline
                    # in tag A is never paced by exp latency
                    for part in range(0, span, 512):
                        n = min(512, span - part)
                        ps_c = psum.tile([B, 512], F32, tag="B", bufs=2,
                                         name="ps_c")
                        nc.tensor.matmul(
                            ps_c[:, 0:n],
                            kT_sb[:, kj * B:(kj + 1) * B],
                            qT_sb[:, h * S + kj * B + part:
                                  h * S + kj * B + part + n],
                            start=True, stop=True,
                        )
                        if part == 0:
                            nc.vector.tensor_add(
                                ps_c[:, 0:B], ps_c[:, 0:B], dmask_sb[:]
                            )
                        nc.scalar.activation(
                            pT[:, _OFFS[kj] + part:_OFFS[kj] + part + n],
                            ps_c[:, 0:n], Exp
                        )
                    return
                ps_sc = psum.tile([B, 1024], F32, tag="A", bufs=2, name="ps_sc")
                for part in range(0, span, 512):
                    n = min(512, span - part)
                    nc.tensor.matmul(
                        ps_sc[:, part:part + n],
                        kT_sb[:, kj * B:(kj + 1) * B],
                        qT_sb[:, h * S + kj * B + part:
                              h * S + kj * B + part + n],
                        start=True, stop=True,
                    )
                # causal bias on the diagonal block
                nc.vector.tensor_add(ps_sc[:, 0:B], ps_sc[:, 0:B], dmask_sb[:])
                nc.scalar.activation(
                    pT[:, _OFFS[kj]:_OFFS[kj] + span], ps_sc[:, 0:span], Exp
                )

            # seq0/seq1 score groups overlap the back half of the qkv phase
            # (their qT/kT inputs are complete by then); seq2 runs after
            chains_done = set()
            sched = {si: [] for si in range(NSB)}
            for kj in range(0, 8):
                sched[8 + kj] = [(h, kj, 8) for h in range(HPC)]
            for kj, si in ((8, 12), (9, 13), (10, 14), (11, 15)):
                sched[si] += [(h, kj, 12) for h in range(HPC)]
            done = set()

            def _chain_head(qi, h, tp_tag="C"):
                    s0, s1 = next(b for b in SEQ_BLOCKS if b[0] <= qi < b[1])
                    if True:
                        pT = pTs[h]
                        ps_pv = psum.tile([B, 129], F32, tag="B", bufs=2)
                        for kj in range(s0, qi + 1):
                            lhsT = pT[:, _OFFS[kj] + (qi - kj) * B:
                                      _OFFS[kj] + (qi - kj + 1) * B]
                            nc.tensor.matmul(
                                ps_pv[:], lhsT,
                                vaug_sb[:, kj * 129:(kj + 1) * 129],
                                start=(kj == s0), stop=(kj == qi),
                            )
                        rc = work.tile([B, 1], F32, tag="rc", bufs=8)
                        nc.vector.reciprocal(rc[:], ps_pv[:, 128:129])
                        at = work.tile([B, B], BF16, tag="at", bufs=6)
                        nc.vector.tensor_scalar_mul(at[:], ps_pv[:, 0:B], rc[:])
                        tp = psum.tile([B, B], BF16, tag=tp_tag, bufs=2)
                        nc.tensor.transpose(tp[:], at[:], ident_sb[:])
                        hi_dst = attnT_hi[:, h, qi * B:(qi + 1) * B]
                        lo_dst = attnT_lo[:, h, qi * B:(qi + 1) * B]
                        nc.scalar.copy(hi_dst, tp[:])
                        nc.vector.tensor_sub(lo_dst, tp[:], hi_dst)

            ot_cur = {}

            def _chain_qi(qi, tp_tag="C"):
                chains_done.add(qi)
                for h in range(HPC):
                    _chain_head(qi, h, tp_tag)

            def _wo(scol, cps=None):
                for cp in (range(NCB) if cps is None else cps):
                    # alternate accumulators across tags C/A (the qkv psum in
                    # A is released by the stage-copy; B stays with chains)
                    if cp % 2 == 1:
                        psoF = psum.tile([B, 512], F32, tag="A", bufs=2,
                                         name="psoA")
                    else:
                        psoF = psum.tile([B, 512], F32, tag="C", bufs=2,
                                         name="psoC")
                    for sub in range(2):
                        pso = psoF[:, sub * 256:(sub + 1) * 256]
                        c0 = scol * 512 + sub * 256
                        nmm = 0
                        for t in range(HPC // 2):
                            for wa, aa in ((woh_sb, attnT_hi),
                                           (wol_sb, attnT_hi),
                                           (woh_sb, attnT_lo)):
                                nc.tensor.matmul(
                                    pso,
                                    wa[:, 2 * t:2 * t + 2, cp * B:(cp + 1) * B],
                                    aa[:, 2 * t:2 * t + 2, c0:c0 + 256],
                                    start=(nmm == 0), stop=(nmm == 5),
                                    perf_mode=DR,
                                )
                                nmm += 1
                    j = cp % 4
                    if scol == 3 and cp >= 28:
                        if j in (0, 2):
                            ot_cur[scol] = work.tile([B, 2, 512], BF16,
                                                     tag="ot1", bufs=2,
                                                     name="ott")
                        ot = ot_cur[scol]
                        if cp % 2 == 0:
                            nc.scalar.copy(ot[:, j % 2, :], psoF[:])
                        else:
                            nc.vector.tensor_copy(ot[:, j % 2, :], psoF[:])
                        if j in (1, 3):
                            nc.sync.dma_start(
                                out=out_h[cp // 4][:, j - 1:j + 1, scol, :],
                                in_=ot[:],
                            )
                        continue
                    if j == 0:
                        ot_cur[scol] = work.tile([B, 4, 512], BF16, tag="ot",
                                                 bufs=2, name="otb")
                    ot = ot_cur[scol]
                    if cp % 2 == 0:
                        nc.scalar.copy(ot[:, j, :], psoF[:])
                    else:
                        nc.vector.tensor_copy(ot[:, j, :], psoF[:])
                    if j == 3:
                        nc.sync.dma_start(
                            out=out_h[cp // 4][:, :, scol, :],
                            in_=ot[:],
                        )

            # =========== Phase A: qkv projection + rope + transposes ===========
            # Per si, the three residual terms split into a hi-part (terms
            # needing only the hi weights: xh@wh, xl@wh) and a lo-part
            # (xh@wl); the psum group opens at the first hi matmul and closes
            # at the last lo matmul. For si 0-3 the parts are emitted in a
            # hand-tuned order interleaved with the DMA stream; si 4-15 are
            # PE-bound and run parts back to back.
            ps_by_si = {}
            CHUNKS = ((512, 768), (0, 256), (256, 512))

            def _qkv_part(si, term, b0, b1):
                if si not in ps_by_si:
                    ps_by_si[si] = psum.tile([B, W768], F32, tag="A", bufs=2,
                                             name=f"ps{si % 2}")
                ps = ps_by_si[si]
                xa = early_xh[si] if term != 2 else early_xl[si]
                wa = wh_sb if term != 3 else wl_sb
                # chunks q0 [0:256] and q1 [256:512] share one 2KB psum
                # zero region: only q0's first matmul carries start=True (its
                # pending-zero covers q1's bytes); giving q1 its own start
                # while q0's group is still open would wipe q0's partials
                for c0, c1 in CHUNKS:
                    for t in range(4 * b0, 4 * b1):
                        nc.tensor.matmul(
                            ps[:, c0:c1],
                            xa[:, 2 * t:2 * t + 2, :],
                            wa[:, 2 * t:2 * t + 2, c0:c1],
                            start=(term == 1 and t == 0 and c0 != 256),
                            stop=(term == 3 and t == NCB // 2 - 1),
                            perf_mode=DR,
                            skip_group_check=True,
                        )

            def _finish_si(si):
                psum_ps = ps_by_si.pop(si)
                cs_t = early_cs.pop(si)
                cs = cs_t[:, 0:256]
                sn = cs_t[:, 256:512]

                # stage the qkv psum to SBUF in one fast Act copy: releases
                # the psum buffer for si+2 immediately, and the strided rope
                # reads below hit SBUF (58-cycle access) instead of PSUM (120)
                ps = work.tile([B, W768], F32, tag="qkvs", bufs=2)
                nc.scalar.copy(ps[:], psum_ps[:])

                # rope on k first (kT feeds the phase-A score groups)
                k_t = work.tile([B, KW], BF16, tag="k", bufs=3)
                ke, ko = ps[:, 512:640:2], ps[:, 513:640:2]
                c64, s64 = cs_t[:, 0:64], cs_t[:, 256:320]
                u1 = work.tile([B, 64], F32, tag="u1", bufs=2)
                u2 = work.tile([B, 64], F32, tag="u2", bufs=2)
                u3 = work.tile([B, 64], F32, tag="u1", bufs=2, name="u3")
                u4 = work.tile([B, 64], F32, tag="u2", bufs=2, name="u4")
                nc.vector.tensor_mul(u1[:], ke, c64)
                nc.vector.tensor_mul(u2[:], ko, s64)
                nc.vector.tensor_sub(k_t[:, 0:KW:2], u1[:], u2[:])
                nc.vector.tensor_mul(u3[:], ke, s64)
                nc.vector.tensor_mul(u4[:], ko, c64)
                nc.vector.tensor_add(k_t[:, 1:KW:2], u3[:], u4[:])

                # v block + ones column (ones = F_W cancels v's host scale)
                nc.scalar.copy(vaug_sb[:, si * 129:si * 129 + 128], ps[:, 640:768])
                nc.vector.memset(vaug_sb[:, si * 129 + 128:si * 129 + 129], F_W)

                # rope on q: [s, d] layout, channels interleaved (even, odd)
                q_t = work.tile([B, QW], BF16, tag="q", bufs=3)
                qe, qo = ps[:, 0:QW:2], ps[:, 1:QW:2]
                t1 = work.tile([B, 256], F32, tag="t1", bufs=2)
                t2 = work.tile([B, 256], F32, tag="t2", bufs=2)
                t3 = work.tile([B, 256], F32, tag="t1", bufs=2, name="t3")
                t4 = work.tile([B, 256], F32, tag="t2", bufs=2, name="t4")
                # even channels on DVE, odd channels concurrently on the
                # otherwise-idle GPSIMD engine (all SBUF-to-SBUF post-stage)
                nc.vector.tensor_mul(t1[:], qe, cs)
                nc.vector.tensor_mul(t2[:], qo, sn)
                nc.vector.tensor_sub(q_t[:, 0:QW:2], t1[:], t2[:])
                nc.gpsimd.tensor_mul(t3[:], qe, sn)
                nc.gpsimd.tensor_mul(t4[:], qo, cs)
                nc.gpsimd.tensor_add(q_t[:, 1:QW:2], t3[:], t4[:])

                # transposes: k first (feeds scores), then q (4 blocks)
                ktp = psum.tile([B, B], BF16, tag="C", bufs=2)
                nc.tensor.transpose(ktp[:], k_t[:], ident_sb[:])
                nc.vector.tensor_copy(kT_sb[:, si * B:(si + 1) * B], ktp[:])
                for h in range(HPC):
                    tp = psum.tile([B, B], BF16, tag="C", bufs=2)
                    nc.tensor.transpose(tp[:], q_t[:, h * B:(h + 1) * B], ident_sb[:])
                    dst = qT_sb[:, h * S + si * B:h * S + (si + 1) * B]
                    if h % 2 == 0:
                        nc.vector.tensor_copy(dst, tp[:])
                    else:
                        nc.scalar.copy(dst, tp[:])

                if si == 7:
                    for h in range(HPC):
                        pT = roam.tile([B, PT_COLS], BF16, tag="roam", bufs=4,
                                       name=f"pT{h}")
                        pTs.append(pT)
                for (h, kj, s1) in sched[si]:
                    _scores(h, kj, s1, chunked=True)
                    done.add((h, kj))
                qi_sched = {12: (0, 4), 13: (1, 5, 8), 14: (2, 6, 9),
                            15: (3, 7)}
                for qi in qi_sched.get(si, ()):
                    _chain_qi(qi)

            # si 0/1: hi-parts interleaved per weight batch, then lo-parts
            # paced by the wl stream, then si2/si3 at full speed
            EARLY_PE = []
            for b in range(4):
                EARLY_PE += [(1, 0, b, b + 1), (2, 0, b, b + 1)]
            for b in range(4):
                EARLY_PE += [(1, 1, b, b + 1), (2, 1, b, b + 1)]
            for b in range(3):
                EARLY_PE += [(3, 0, b, b + 1), (3, 1, b, b + 1)]
            EARLY_PE += [(3, 0, 3, 4), ("fin", 0), (3, 1, 3, 4), ("fin", 1)]
            for ent in EARLY_PE:
                if ent[0] == "fin":
                    _finish_si(ent[1])
                else:
                    _qkv_part(ent[1], ent[0], ent[2], ent[3])
            for si in (2, 3):
                for term in (1, 2, 3):
                    _qkv_part(si, term, 0, 4)
                _finish_si(si)

            for si in range(4, NSB):
                t_ = work.tile([B, NCB, B], F8, tag="xh", bufs=3)
                nc.sync.dma_start(out=t_[:], in_=xh_h[si].bitcast(F8))
                early_xh[si] = t_
                t_ = work.tile([B, NCB, B], F8, tag="xl", bufs=3)
                nc.sync.dma_start(out=t_[:], in_=xl_h[si].bitcast(F8))
                early_xl[si] = t_
                t_ = work.tile([B, 512], BF16, tag="cs", bufs=3)
                nc.sync.dma_start(out=t_[:], in_=cs_h[si])
                early_cs[si] = t_
                for term in (1, 2, 3):
                    _qkv_part(si, term, 0, 4)
                _finish_si(si)

            # wo reuses the wqkv slots; attnT is its own tensor so chains can
            # write it before the last qkv matmul retires
            woh_sb = bigp.tile([B, HPC, DIM], F8, tag="bigh")
            wol_sb = bigp.tile([B, HPC, DIM], F8, tag="bigl")
            for g in range(4):
                g0, g1 = g * 1024, (g + 1) * 1024
                for t in range(HPC // 2):
                    nc.sync.dma_start(out=woh_sb[:, 2 * t:2 * t + 2, g0:g1],
                                      in_=woh_h[t][:, :, g0:g1].bitcast(F8))
                for t in range(HPC // 2):
                    nc.sync.dma_start(out=wol_sb[:, 2 * t:2 * t + 2, g0:g1],
                                      in_=wol_h[t][:, :, g0:g1].bitcast(F8))

            # ===== Phase B2: PV + normalize, interleaved with wo per scol =====
            # seq0/seq1 chains (except 10/11) ran inside phase A; the
            # latency-bound leftover chains and seq2 scores/chains interleave
            # with wo(0)'s dense cp groups, covering the wo weight stream
            units = []
            for h in range(HPC):
                units.append(("sc", 12, h))
            for h in range(HPC):
                units.append(("sc", 13, h))
                units.append(("ch", 10, h))
            for h in range(HPC):
                units.append(("sc", 14, h))
                units.append(("ch", 11, h))
            for h in range(HPC):
                units.append(("sc", 15, h))
            for qi in (12, 13, 14, 15):
                for h in range(HPC):
                    units.append(("ch", qi, h))
            cps = [(s, c) for s in range(3) for c in range(NCB)]
            nu, ncp = len(units), len(cps)
            ci = 0
            RAMP = 10   # no cps among the first units: their wo weights are
                       # still streaming and a blocked cp stalls the in-order
                       # PE queue behind it
            for i, u in enumerate(units):
                if u[0] == "ch":
                    _chain_head(u[1], u[2], tp_tag="B")
                else:
                    if (u[2], u[1]) not in done:
                        _scores(u[2], u[1], 16)
                if i < RAMP:
                    continue
                take = ((i + 1 - RAMP) * ncp) // (nu - RAMP) - ((i - RAMP) * ncp) // (nu - RAMP)
                for s, c in cps[ci:ci + take]:
                    _wo(s, [c])
                ci += take
            for s, c in cps[ci:]:
                _wo(s, [c])
            _wo(3)

    nc.finalize()
    return nc


def get_program():
    global _PROGRAM
    if _PROGRAM is None:
        _PROGRAM = _build_program()
    return _PROGRAM


def make_in_maps(x, cos, sin, wq, wk, wv, wo):
    import ml_dtypes
    f8 = ml_dtypes.float8_e4m3fn

    def split8(v):
        hi = v.astype(f8)
        lo = (v - hi.astype(np.float32)).astype(f8)
        return hi, lo

    x = np.asarray(x, np.float32)
    cos = np.asarray(cos, np.float32)
    sin = np.asarray(sin, np.float32)
    wq = np.asarray(wq, np.float32)
    wk = np.asarray(wk, np.float32)
    wv = np.asarray(wv, np.float32)
    wo = np.asarray(wo, np.float32)

    # xt[si, p, cb, s] = x[si*B + s, cb*B + p]
    xt = np.ascontiguousarray(
        x.reshape(NSB, B, NCB, B).transpose(0, 3, 2, 1)
    )
    xh, xl = split8(xt)
    # cos||sin tiled 4x along channels (per-head repeat), blocked by si,
    # divided by 512 to cancel the 512*sqrt(scale) on wq/wk (leaving sqrt(scale))
    cosr = np.tile(cos / 512.0, (1, HPC)).reshape(NSB, B, 2 * HEAD_DIM)
    sinr = np.tile(sin / 512.0, (1, HPC)).reshape(NSB, B, 2 * HEAD_DIM)
    import ml_dtypes as _mld
    csr = np.ascontiguousarray(
        np.concatenate([cosr, sinr], axis=2).astype(_mld.bfloat16))
    # diagonal-block causal bias in scoresT layout: allow sq >= sk
    i = np.arange(B)
    dmask = np.where(i[None, :] >= i[:, None], 0.0, -30000.0).astype(np.float32)
    ident = np.eye(B, dtype=np.float32).astype(ml_dtypes.bfloat16)

    W768 = QW + 2 * KW
    in_maps = []
    for c in range(N_CORES):
        wq_c = wq[:, c * QW:(c + 1) * QW] * F_W
        wk_c = wk[:, c * KW:(c + 1) * KW] * F_W
        wv_c = wv[:, c * KW:(c + 1) * KW] * F_W
        wqkv_c = np.concatenate([wq_c, wk_c, wv_c], axis=1)
        # blocked [b, p, u, j] = wqkv[(8b+u)*B + p, j]
        wqkv_b = np.ascontiguousarray(
            wqkv_c.reshape(NCB // 8, 8, B, W768).transpose(0, 2, 1, 3))
        whb, wlb = split8(wqkv_b)
        wo_c = wo[c * QW:(c + 1) * QW, :] * E_WO
        wo_b = np.ascontiguousarray(
            wo_c.reshape(HPC // 2, 2, B, DIM).transpose(0, 2, 1, 3))
        woh, wol = split8(wo_b)
        in_maps.append({
            "xh": xh.view(np.uint8),
            "xl": xl.view(np.uint8),
            "whb": whb.view(np.uint8),
            "wlb": wlb.view(np.uint8),
            "woh": woh.view(np.uint8),
            "wol": wol.view(np.uint8),
            "csr": csr,
            "dmask": dmask,
            "ident": ident,
        })
    return in_maps


def combine_outputs(results):
    acc = np.zeros((NCB // 4, B, 4, 4, 512), np.float32)
    for r in results:
        acc += np.asarray(r["outp"]).astype(np.float32)
    acc *= 1.0 / E_WO
    # [cpg, p, j, scol, col] -> [cpg, j, p, scol, col] -> [DIM, S] -> [S, DIM]
    full = acc.transpose(0, 2, 1, 3, 4).reshape(DIM, S)
    return np.ascontiguousarray(full.T)


def kernel(x, cos, sin, mask, wq, wk, wv, wo):
    nc = get_program()
    in_maps = make_in_maps(x, cos, sin, wq, wk, wv, wo)
    res = run_bass_kernel_spmd(nc, in_maps, core_ids=list(range(N_CORES)))
    return combine_outputs(res.results)
